# revision 53
# baseline (speedup 1.0000x reference)
"""AttentiveMLP GNN message-passing kernel for 8 Trainium2 NeuronCores.

Sharding: edges are partitioned BY DESTINATION NODE (each core owns ~N/8 nodes
plus all their incoming edges) so no cross-core collectives are needed. Nodes
are grouped on the host into degree-bucket classes (a pure layout/permutation
choice); within a class every node has exactly d edge slots (pad slots carry
logit -60 -> weight ~0), so segment softmax and the attention-weighted
aggregation are static dense ops over [128, T*d] tiles.

Key device-side structure (v2, tuned off the HW perfetto trace):
 - softmax max-subtraction is dropped (logits ~N(0,1): exp() cannot overflow,
   result is mathematically identical), killing two full edge passes.
 - alpha = exp(lg) * (1/den) is folded BEFORE the edge-feature multiply, so
   the aggregation tree output needs no post-normalization pass.
 - edge features ride in pair-major layout (t, s2, f, s1) with s1 a 2-slot
   pair: every level of the pairwise-halving reduction tree is a bf16
   tensor_tensor add over 32-element contiguous runs (the HW DVE only hits
   its 2x 16-bit mode on packed runs; the old feature-major layout decayed
   to 1x on deep levels), and the tree STOPS at slot-pairs: the 32x32 stream
   transpose lands pair partials on adjacent aggT rows (2f, 2f+1) and the
   ctx matmul's wet4 carries W_et on both rows, absorbing the final add into
   the PE contraction for free (the old layout wasted rows 16-31 on
   zero-weight duplicates).
 - ELU is computed as elu(x)+1 = min(exp(x),1) + relu(x) (two activations +
   one fused scalar_tensor_tensor); the +1 shift is folded into b1 on host.
 - logits ride bf16 as ONE merged [128, sum(T*d)] tensor (single descriptor
   gen, exps queue at the head of the Act engine); bf16 consts ride as one
   merged [128, 512] tensor.
 - classes are processed LARGEST FIRST so the final MLP blocks only wait on
   a tiny class's tree at the tail; the d=12 class tree rides GpSimd to
   offload the Vector engine.
 - MLP chunk g=3 feeds the PE from base partition 96 directly (legal when
   stationary and moving share the base), killing the cb3 copies.
 - scheduling: all exps are queued on the Act engine upfront; the class loop
   is software-pipelined (stage1 = den/recip/alpha/prod/tree of class i+1
   issues before stage2 = transpose of class i); each ready block's ELU head
   issues one block ahead of its MLP chunk loop; the last blocks alternate
   h/ov between Act and Vector (tensor_scalar add+max).
"""
import os
import numpy as np
import ml_dtypes
from contextlib import ExitStack

import concourse.bass as bass
import concourse.bacc as bacc
import concourse.tile as tile
import concourse.mybir as mybir
from concourse.bass_utils import run_bass_kernel_spmd

N_NODES = 100000
N_EDGES = 1600000
EF = 16
HID = 32
NF = 128
NCORES = 8
CHUNK = 512

f32 = mybir.dt.float32
bf16 = mybir.dt.bfloat16
BF = ml_dtypes.bfloat16

PAD_LG = -60.0  # exp(-60) ~ 8.8e-27: pad slots contribute ~0 weight, no inf/nan

BUCKETS = [8, 12, 14, 16, 18, 20, 24, 40,
           64, 128, 256, 1024, 4096, 65536, 1048576, 2097152]

# classes whose reduction tree runs on GpSimd (to offload Vector); these are
# scheduled FIRST so later V-side stage1 work covers the GpSimd latency
GP_TREE_D = ()
# number of trailing blocks whose h/ov relus alternate onto Vector
V_RELU_BLOCKS = 2
# priority offset for block-drain compute (transposes, ELU, MLP): the tile
# list scheduler otherwise prefers older stage1 work and delays the unlock
# of downstream engines
PRIO_OFF = 1_000_000


def _bin_blocks(n):
    """Contiguous ranges of power-of-2 width covering [0, n), descending."""
    out, s = [], 0
    for k in range(21, -1, -1):
        w = 1 << k
        if n & w:
            out.append((s, w))
            s += w
    return out


def _bucket_of(deg):
    b = np.zeros_like(deg)
    nz = deg > 0
    idx = np.searchsorted(np.asarray(BUCKETS), deg[nz])
    b[nz] = np.asarray(BUCKETS)[idx]
    return b


def _build_plan(dst):
    deg = np.bincount(dst, minlength=N_NODES)
    deg = _bucket_of(deg)
    order = np.argsort(deg, kind="stable")
    sdeg = deg[order]
    uniq, starts, counts = np.unique(sdeg, return_index=True, return_counts=True)
    ncls = len(uniq)
    rank = np.arange(N_NODES) - np.repeat(starts, counts)
    dev = rank % NCORES
    row_in_class = rank // NCORES
    n_pad = (counts + NCORES - 1) // NCORES
    n_pad = ((n_pad + 127) // 128) * 128   # 128-aligned class rows/offsets

    # Order: the tiniest class first (its ef lands within ~1us, so the whole
    # exp->den->alpha->prod->tree->transpose->ELU chain warms up the MLP
    # pipeline while the big ef transfers stream), then the cheapest class
    # that covers a full 16-tile block, then the rest LARGEST-work first so
    # the tail only waits on tiny classes.
    cls_ids = [ci for ci in range(ncls) if uniq[ci] > 0]
    cls_ids.sort(key=lambda ci: -int(uniq[ci]) * int(n_pad[ci]))
    head = []
    if cls_ids:
        tiny = cls_ids[-1]
        cls_ids.remove(tiny)
        head.append(tiny)
    starters = [ci for ci in cls_ids if n_pad[ci] >= 16 * 128]
    if starters:
        first = min(starters, key=lambda ci: int(uniq[ci]) * int(n_pad[ci]))
        cls_ids.remove(first)
        head.append(first)
    cls_ids = head + cls_ids
    if uniq[0] == 0:
        cls_ids = cls_ids + [0]
    offs_arr = np.zeros(ncls, dtype=np.int64)
    acc = 0
    for ci in cls_ids:
        offs_arr[ci] = acc
        acc += n_pad[ci]
    R = int(acc)

    cls_of_pos = np.repeat(np.arange(ncls), counts)
    lrow = offs_arr[cls_of_pos] + row_in_class

    node_dev = np.empty(N_NODES, dtype=np.int64)
    node_lrow = np.empty(N_NODES, dtype=np.int64)
    node_dev[order] = dev
    node_lrow[order] = lrow

    classes = [(int(uniq[ci]), int(n_pad[ci]), int(offs_arr[ci])) for ci in cls_ids]
    deg0_rows = classes[-1][1] if classes and classes[-1][0] == 0 else 0
    kclasses = [c for c in classes if c[0] > 0]
    zero_tail_start = R - deg0_rows

    n_tiles = R // 128
    # block b covers tiles [t0b, t0b+ntb); chunk g of block b is 32*ntb wide.
    # A small leading block matching the tiny first class lets the MLP
    # pipeline start while the big ef transfers are still streaming.
    blocks = []
    start = 0
    t_first = (kclasses[0][1] + 127) // 128 if kclasses else 0
    if 0 < t_first < 16:
        blocks.append((0, t_first))
        start = t_first
    while start + 16 <= n_tiles:
        blocks.append((start, 16))
        start += 16
    if start < n_tiles:
        blocks.append((start, n_tiles - start))

    return dict(
        uniq=uniq, counts=counts, node_dev=node_dev, node_lrow=node_lrow,
        R=R, n_tiles=n_tiles, blocks=blocks, kclasses=kclasses,
        zero_tail_start=zero_tail_start,
    )


def _mlpcol(r, blocks):
    """node row -> column in the transposed-MLP [NF, R] layout."""
    r = np.asarray(r)
    t = r // 128
    q = (r % 128) // 32
    c = r % 32
    t0s = np.array([t0b for (t0b, ntb) in blocks])
    nts = np.array([ntb for (t0b, ntb) in blocks])
    cbase = np.concatenate([[0], np.cumsum(4 * 32 * nts)])[:-1]
    b = np.searchsorted(t0s, t, side="right") - 1
    W = 32 * nts[b]
    return cbase[b] + q * W + 32 * (t - t0s[b]) + c


def _shard_inputs(inputs, plan):
    lg = np.ascontiguousarray(
        np.asarray(inputs["edge_logits"], dtype=np.float32).reshape(-1))
    ef = np.ascontiguousarray(np.asarray(inputs["edge_feats"], dtype=np.float32))
    nf = np.asarray(inputs["node_feats"], dtype=np.float32)
    dst = np.asarray(inputs["dst"])
    W_et = np.asarray(inputs["W_et"], dtype=np.float32)
    b_et = np.asarray(inputs["b_et"], dtype=np.float32)
    W1 = np.asarray(inputs["W1"], dtype=np.float32)
    b1 = np.asarray(inputs["b1"], dtype=np.float32)
    W2 = np.asarray(inputs["W2"], dtype=np.float32)
    b2 = np.asarray(inputs["b2"], dtype=np.float32)

    node_dev, node_lrow = plan["node_dev"], plan["node_lrow"]
    R, blocks = plan["R"], plan["blocks"]
    kclasses = plan["kclasses"]

    ekey = node_dev[dst] * R + node_lrow[dst]
    eorder = np.argsort(ekey, kind="stable")
    sk = ekey[eorder]
    newrun = np.empty(N_EDGES, dtype=bool)
    newrun[0] = True
    newrun[1:] = sk[1:] != sk[:-1]
    runstart = np.maximum.accumulate(np.where(newrun, np.arange(N_EDGES), 0))
    slot = np.arange(N_EDGES) - runstart
    e_dev = node_dev[dst[eorder]]
    e_lrow = node_lrow[dst[eorder]]
    lg_s = lg[eorder].astype(BF)
    ef_s = ef[eorder].astype(BF)

    lg_offs = []
    acc = 0
    for (d, npad, off) in kclasses:
        T = (npad + 127) // 128
        lg_offs.append(acc)
        acc += T * d
    lg_total = acc

    in_maps = [dict() for _ in range(NCORES)]
    for dv in range(NCORES):
        dmask = e_dev == dv
        d_lrow = e_lrow[dmask]
        d_slot = slot[dmask]
        d_lg = lg_s[dmask]
        d_ef = ef_s[dmask]
        lgall = np.full((128, lg_total), PAD_LG, dtype=BF)
        for idx, (d, npad, off) in enumerate(kclasses):
            T = (npad + 127) // 128
            cmask = (d_lrow >= off) & (d_lrow < off + npad)
            r = d_lrow[cmask] - off
            s = d_slot[cmask]
            p = r % 128
            t = r // 128
            # logits: [p, lg_offs + t*d + s] (slot s contiguous per tile)
            lgall[p, lg_offs[idx] + t * d + s] = d_lg[cmask]  # noqa (split below)
            # pair-major feature slots: s = 2*s2 + s1 ->
            #   [p, (t*(d//2) + s2)*2*EF + 2*f + s1]
            flat_ef = np.zeros((128, T * d * EF), dtype=BF)
            col = ((t * (d // 2) + s // 2) * 2 * EF + (s % 2))[:, None] \
                + np.arange(EF)[None, :] * 2
            flat_ef[p[:, None], col] = d_ef[cmask]
            in_maps[dv][f"ef{idx}"] = flat_ef
        # class-0 logits ride their own small DMA so the first exp can start
        # as early as possible; the rest follow in one transfer
        c0 = lg_offs[1] if len(kclasses) > 1 else lg_total
        in_maps[dv]["lg0"] = np.ascontiguousarray(lgall[:, :c0])
        in_maps[dv]["lgrest"] = np.ascontiguousarray(lgall[:, c0:])

    for dv in range(NCORES):
        sel = node_dev == dv
        nid = np.nonzero(sel)[0]
        lr = node_lrow[sel]
        nf_dev = np.zeros((R, NF), dtype=np.float32)
        nf_dev[_mlpcol(lr, blocks)] = nf[nid]
        in_maps[dv]["nfT"] = np.ascontiguousarray(nf_dev.T).astype(BF)

    # device computes cb' = elu(ctx)+1 = min(exp(z),1)+relu(z); fold the -1
    # correction into b1: h = relu(W1c^T cb' + W1n^T nf + (b1 - colsum(W1c)))
    b1 = b1 - W1[:HID].sum(axis=0)
    # wet4 rows carry W_et on BOTH pair rows (2f, 2f+1): the ctx matmul sums
    # the two slot-pair partials the transpose lands on adjacent rows.
    wet4 = np.zeros((128, 128), dtype=BF)
    bet4 = np.zeros((128, 1), dtype=np.float32)
    for g in range(4):
        for s1 in range(2):
            wet4[32 * g + s1:32 * g + 2 * EF + s1:2, 32 * g:32 * g + HID] = \
                W_et.astype(BF)
        bet4[32 * g:32 * g + HID, 0] = b_et
    cb16 = np.zeros((128, 512), dtype=BF)
    cb16[:, 0:128] = wet4
    cb16[:, 128:256] = np.tile(W1[:HID], (4, 1)).astype(BF)
    cb16[:, 256:384] = W1[HID:].astype(BF)
    cb16[:, 384:512] = W2.astype(BF)
    cf32 = np.zeros((128, 3), dtype=np.float32)
    cf32[:, 0:1] = bet4
    cf32[:, 1:2] = b1.reshape(NF, 1)
    cf32[:, 2:3] = b2.reshape(NF, 1)
    for dv in range(NCORES):
        in_maps[dv]["cb16"] = cb16.copy()
        in_maps[dv]["cf32"] = cf32.copy()
    return in_maps


def _unshard(results, plan):
    node_dev, node_lrow = plan["node_dev"], plan["node_lrow"]
    blocks = plan["blocks"]
    out = np.empty((N_NODES, NF), dtype=np.float32)
    for dv in range(NCORES):
        sel = node_dev == dv
        nid = np.nonzero(sel)[0]
        lr = node_lrow[sel]
        out_dev = results[dv]["outT"].T.astype(np.float32)
        out[nid] = out_dev[_mlpcol(lr, blocks)]
    return out


def _build_kernel(plan):
    kclasses = plan["kclasses"]
    R = plan["R"]
    blocks = plan["blocks"]
    n_blocks = len(blocks)
    zts = plan["zero_tail_start"]

    nc = bacc.Bacc("TRN2", target_bir_lowering=False, debug=False,
                   num_devices=NCORES)

    lg_offs = []
    acc = 0
    for (d, npad, off) in kclasses:
        T = (npad + 127) // 128
        lg_offs.append(acc)
        acc += T * d
    lg_total = acc

    ef_d = []
    for idx, (d, npad, off) in enumerate(kclasses):
        T = (npad + 127) // 128
        ef_d.append(nc.dram_tensor(f"ef{idx}", [128, T * d * EF], bf16,
                                   kind="ExternalInput"))
    c0 = lg_offs[1] if len(kclasses) > 1 else lg_total
    lg0_d = nc.dram_tensor("lg0", [128, c0], bf16, kind="ExternalInput")
    lgrest_d = nc.dram_tensor("lgrest", [128, lg_total - c0], bf16,
                              kind="ExternalInput")
    nfT_d = nc.dram_tensor("nfT", [NF, R], bf16, kind="ExternalInput")
    cb16_d = nc.dram_tensor("cb16", [128, 512], bf16, kind="ExternalInput")
    cf32_d = nc.dram_tensor("cf32", [128, 3], f32, kind="ExternalInput")
    out_d = nc.dram_tensor("outT", [NF, R], bf16, kind="ExternalOutput")

    # column base of each block in the [NF, R] layout
    col_base = []
    acc = 0
    for (t0b, ntb) in blocks:
        col_base.append(acc)
        acc += 4 * 32 * ntb
    assert acc == R

    with tile.TileContext(nc) as tc, ExitStack() as ctx:
        const_pool = ctx.enter_context(tc.tile_pool(name="const", bufs=1))
        agg_pool = ctx.enter_context(tc.tile_pool(name="agg", bufs=1))
        cls_pool = ctx.enter_context(tc.tile_pool(name="cls", bufs=4))
        work_pool = ctx.enter_context(tc.tile_pool(name="work", bufs=3))
        tree_pool = ctx.enter_context(tc.tile_pool(name="tree", bufs=3))
        small_pool = ctx.enter_context(tc.tile_pool(name="small", bufs=4))
        mlp_pool = ctx.enter_context(tc.tile_pool(name="mlp", bufs=3))
        ctx_pool = ctx.enter_context(tc.tile_pool(name="ctxs", bufs=2))
        ctx_psum = ctx.enter_context(tc.tile_pool(name="ctxp", bufs=3, space="PSUM"))
        mlp1_psum = ctx.enter_context(tc.tile_pool(name="m1p", bufs=3, space="PSUM"))
        mlp2_psum = ctx.enter_context(tc.tile_pool(name="m2p", bufs=2, space="PSUM"))

        # Two logits DMAs (class 0's small slice lands first so exp0 starts
        # ASAP) and all exps queued at the head of the Act engine's in-order
        # queue.
        lgt = cls_pool.tile([128, lg_total], bf16, name="lgall")
        nc.sync.dma_start(lgt[0:128, 0:c0], lg0_d.ap())
        nc.sync.dma_start(lgt[0:128, c0:lg_total], lgrest_d.ap())
        x_tiles = []
        for ci, (dc, npadc, _) in enumerate(kclasses):
            Tc = (npadc + 127) // 128
            xt = cls_pool.tile([128, Tc * dc], bf16, tag=f"x{ci}", name=f"x{ci}")
            nc.scalar.activation(xt[:], lgt[0:128, lg_offs[ci]:lg_offs[ci] + Tc * dc],
                                 mybir.ActivationFunctionType.Exp)
            x_tiles.append(xt)
        prefetched = {}

        def prefetch_ef(pi):
            if pi >= len(kclasses) or pi in prefetched:
                return
            dp, npadp, _ = kclasses[pi]
            Tp = (npadp + 127) // 128
            eftp = cls_pool.tile([128, Tp * dp * EF], bf16, tag="ef", name=f"eft{pi}")
            nc.sync.dma_start(eftp[:], ef_d[pi].ap())
            prefetched[pi] = eftp

        prefetch_ef(0)
        prefetch_ef(1)

        consts = const_pool.tile([128, 512], bf16, name="cb16c")
        nc.gpsimd.dma_start(consts[:], cb16_d.ap())
        constf = const_pool.tile([128, 3], f32, name="cf32c")
        nc.gpsimd.dma_start(constf[:], cf32_d.ap())
        wet4 = consts[0:128, 0:128]
        w1c = consts[0:128, 128:256]
        w1n = consts[0:128, 256:384]
        w2 = consts[0:128, 384:512]
        bet4 = constf[0:128, 0:1]
        b1 = constf[0:128, 1:2]
        b2 = constf[0:128, 2:3]

        aggT_blocks = []
        for b, (t0b, ntb) in enumerate(blocks):
            ab = agg_pool.tile([128, 32 * ntb], bf16, name=f"aggT{b}")
            aggT_blocks.append(ab)
        # deg0 tail rows (none for this input, but keep correct in general):
        # their aggT columns are never written by any class; zero them so the
        # ctx matmul sees context 0 (b_et==0 -> elu path produces cb'=1).
        if zts < R:
            t_lo = zts // 128
            for b, (t0b, ntb) in enumerate(blocks):
                lo = max(t0b, t_lo)
                hi = t0b + ntb
                if lo < hi:
                    nc.gpsimd.memset(
                        aggT_blocks[b][0:128, 32 * (lo - t0b):32 * (hi - t0b)], 0.0)

        # ---------------- Phase B+C: ELU + MLP per block ----------------
        nf_tiles = {}

        def prefetch_nf(b):
            if b >= n_blocks or b in nf_tiles:
                return
            t0b, ntb = blocks[b]
            W = 32 * ntb
            cb4 = col_base[b]
            t = mlp_pool.tile([NF, 4 * W], bf16, tag="nfblk", name=f"nfblk{b}")
            nc.sync.dma_start(t[:], nfT_d.ap()[:, cb4:cb4 + 4 * W])
            nf_tiles[b] = t

        cb_tiles = {}

        def emit_elu(b):
            t0b, ntb = blocks[b]
            W = 32 * ntb
            with tc.high_priority(offset=PRIO_OFF):
                ctx4 = ctx_psum.tile([128, W], f32, tag="ctx4", name=f"ctx4_{b}")
                nc.tensor.matmul(ctx4[:], wet4, aggT_blocks[b][:])
                s1 = mlp_pool.tile([128, W], bf16, tag="s1", name=f"s1_{b}")
                if b >= n_blocks - V_RELU_BLOCKS:
                    nc.vector.tensor_scalar(s1[:], ctx4[:], bet4, 0.0,
                                            mybir.AluOpType.add,
                                            mybir.AluOpType.max)
                else:
                    nc.scalar.activation(s1[:], ctx4[:],
                                         mybir.ActivationFunctionType.Relu,
                                         bias=bet4, scale=1.0)
                s2 = mlp_pool.tile([128, W], bf16, tag="s2", name=f"s2_{b}")
                nc.scalar.activation(s2[:], ctx4[:],
                                     mybir.ActivationFunctionType.Exp,
                                     bias=bet4, scale=1.0)
            # cb' = elu(ctx)+1 = min(exp(z),1) + relu(z); the +1 shift is
            # compensated in b1 on the host. s1/s2 ride bf16 so the stst
            # qualifies for the DVE's 2x 16-bit mode.
            cb = ctx_pool.tile([128, W], bf16, tag="cb", name=f"cb_{b}")
            with tc.high_priority(offset=PRIO_OFF):
                nc.vector.scalar_tensor_tensor(cb[:], s2[:], 1.0, s1[:],
                                               mybir.AluOpType.min,
                                               mybir.AluOpType.add)
            cb_tiles[b] = cb
            prefetch_nf(b)
            prefetch_nf(b + 1)

        def emit_mlp(b):
            t0b, ntb = blocks[b]
            W = 32 * ntb
            cb4 = col_base[b]
            cb = cb_tiles.pop(b)
            nfblk = nf_tiles.pop(b)
            oblk = mlp_pool.tile([NF, 4 * W], bf16, tag="oblk", name=f"oblk{b}")
            last = b == n_blocks - 1
            with tc.high_priority(offset=PRIO_OFF):
                for g in range(4):
                    j = 4 * b + g
                    nfb = nfblk[0:NF, g * W:(g + 1) * W]
                    ps1 = mlp1_psum.tile([NF, W], f32, tag="ps1", name=f"ps1_{j}")
                    cbg = cb[32 * g:32 * (g + 1), :]
                    w1cg = w1c[32 * g:32 * (g + 1), 0:NF]
                    nc.tensor.matmul(ps1[:], w1n, nfb, start=True, stop=False)
                    # base partition 96 is legal for a 32-row tile but the
                    # default inference path refuses it; pass it explicitly
                    nc.tensor.matmul(ps1[:], w1cg, cbg, start=False, stop=True,
                                     tile_position=(32 * g, 0))
                    h = mlp_pool.tile([NF, W], bf16, tag="h", name=f"h{j}")
                    tail = b >= n_blocks - V_RELU_BLOCKS
                    if tail and j % 2 == 1:
                        nc.vector.tensor_scalar(h[:], ps1[:], b1, 0.0,
                                                mybir.AluOpType.add,
                                                mybir.AluOpType.max)
                    else:
                        nc.scalar.activation(h[:], ps1[:],
                                             mybir.ActivationFunctionType.Relu,
                                             bias=b1, scale=1.0)
                    ps2 = mlp2_psum.tile([NF, W], f32, tag="ps2", name=f"ps2_{j}")
                    nc.tensor.matmul(ps2[:], w2, h[:])
                    ov = oblk[0:NF, g * W:(g + 1) * W]
                    if tail and j % 2 == 0:
                        nc.vector.tensor_scalar(ov, ps2[:], b2, 0.0,
                                                mybir.AluOpType.add,
                                                mybir.AluOpType.max)
                    else:
                        nc.scalar.activation(ov, ps2[:],
                                             mybir.ActivationFunctionType.Relu,
                                             bias=b2, scale=1.0)
            nc.gpsimd.dma_start(out_d.ap()[:, cb4:cb4 + 2 * W],
                                oblk[0:NF, 0:2 * W])
            nc.gpsimd.dma_start(out_d.ap()[:, cb4 + 2 * W:cb4 + 4 * W],
                                oblk[0:NF, 2 * W:4 * W])

        next_block = [0]

        # ------------- Phase A: per-class segment softmax + aggregation ------
        # Two-stage software pipeline: stage1 (den/recip/alpha/prod/tree) of
        # class i+1 is ISSUED before stage2 (transpose/emit) of class i, so
        # the in-order Vector queue always holds independent work ahead of
        # any op that waits on a GpSimd tree.
        stage_state = {}

        # The LAST class's prod+tree ride GpSimd (chunked so the
        # high-priority alphas still preempt): its den is hoisted early by
        # the scheduler, GpSimd computes it during its idle mid-window, and
        # the Vector engine reaches the final transposes sooner.
        n_cls = len(kclasses)

        def on_gpsimd(idx):
            return n_cls >= 4 and idx == n_cls - 1

        alphas = {}

        def prep(idx):
            """den -> recip -> alpha for one class (idempotent)."""
            if idx in alphas:
                return alphas[idx]
            d, npad, off = kclasses[idx]
            T = (npad + 127) // 128
            xt = x_tiles[idx]
            x3 = xt[:].rearrange("p (t d) -> p t d", t=T)
            # den[p,t] = sum_d x; alpha = x * (1/den) pre-normalizes so the
            # tree output feeds the transpose directly. alpha rides the
            # otherwise-idle GpSimd (SBUF-only op); recip is DVE-only.
            den = small_pool.tile([128, T], f32, tag="den", name=f"den{idx}")
            nc.vector.tensor_reduce(den[:], x3, mybir.AxisListType.X,
                                    mybir.AluOpType.add)
            rd = small_pool.tile([128, T], f32, tag="rd", name=f"rd{idx}")
            nc.vector.reciprocal(rd[:], den[:])
            atag = f"gal{idx}" if on_gpsimd(idx) else "al"
            alpha = small_pool.tile([128, T * d], bf16, tag=atag, name=f"al{idx}")
            alpha3 = alpha[:].rearrange("p (t d) -> p t d", t=T)
            rd_b = rd[:].unsqueeze(2).broadcast_to([128, T, d])
            # class 0 is the latency-critical warmup chain: keep its alpha on
            # the DVE (no cross-engine hop); later classes ride GpSimd at
            # high priority so queued cb work never delays the prod chain
            if idx == 0:
                nc.vector.tensor_tensor(alpha3, x3, rd_b, mybir.AluOpType.mult)
            else:
                with tc.high_priority(offset=PRIO_OFF):
                    nc.gpsimd.tensor_tensor(alpha3, x3, rd_b,
                                            mybir.AluOpType.mult)
            alphas[idx] = alpha
            return alpha

        def stage1(idx):
            d, npad, off = kclasses[idx]
            d2 = d // 2
            T = (npad + 127) // 128
            prefetch_ef(idx + 2)
            eft = prefetched[idx]
            alpha = prep(idx)

            # prod[p, ts2, f, s1] = ef * alpha  (bf16 2x mode: packed pairs).
            # A small leading tile-slice rides GpSimd (slow but idle) so the
            # Vector engine finishes stage1 sooner.
            ptag = f"gprod{idx}" if on_gpsimd(idx) else "prod"
            prod = work_pool.tile([128, T * d * EF], bf16, tag=ptag, name=f"prod{idx}")
            ef4 = eft[:].rearrange("p (ts f s1) -> p ts f s1", f=EF, s1=2)
            al4 = alpha[:].rearrange("p (ts s1) -> p ts s1", s1=2) \
                .unsqueeze(2).broadcast_to([128, T * d2, EF, 2])
            prod4 = prod[:].rearrange("p (ts f s1) -> p ts f s1", f=EF, s1=2)
            gp = on_gpsimd(idx)

            def emit_tt(out_ap, a_ap, b_ap, n1, per_unit_free,
                        op=mybir.AluOpType.add):
                """tensor_tensor on V, or on G chunked along dim 1 so
                high-priority alphas can preempt between pieces."""
                if not gp:
                    nc.vector.tensor_tensor(out_ap, a_ap, b_ap, op)
                    return
                gsz = max(1, 640 // per_unit_free)
                nd = len(out_ap.shape)
                s = 0
                while s < n1:
                    e = min(n1, s + gsz)
                    if nd == 4:
                        sl = (slice(None), slice(s, e), slice(None), slice(None))
                    else:
                        sl = (slice(None), slice(s, e), slice(None))
                    nc.gpsimd.tensor_tensor(out_ap[sl], a_ap[sl], b_ap[sl], op)
                    s = e

            emit_tt(prod4, al4, ef4, T * d2, 32, mybir.AluOpType.mult)

            # pairwise-halving tree over s2 (32-elem contiguous runs at every
            # level); STOPS at slot-pairs -- the ctx matmul absorbs the last
            # add via the doubled wet4 rows.
            prodc = prod[:].rearrange("p (t s c) -> p t s c", t=T, c=32)
            agg_parts = []
            for bi, (s0, w) in enumerate(_bin_blocks(d2)):
                cur = prodc[:, :, s0:s0 + w, :]
                cw = w
                while cw > 1:
                    half = cw // 2
                    ttag = (f"gag{idx}_{bi}_{half}" if on_gpsimd(idx)
                            else f"ag{bi}_{half}")
                    nt = tree_pool.tile([128, T * half * 32], bf16,
                                        tag=ttag, name=f"ag{idx}_{bi}_{half}")
                    nt4 = nt[:].rearrange("p (t s c) -> p t s c", t=T, c=32)
                    emit_tt(nt4, cur[:, :, 0:half, :], cur[:, :, half:cw, :],
                            T, 32 * half)
                    cur = nt4
                    cw = half
                agg_parts.append(cur.rearrange("p t s c -> p t (s c)"))
            a16u = agg_parts[0]
            for k in range(1, len(agg_parts)):
                mtag = f"gam{idx}_{k}" if on_gpsimd(idx) else f"am{k}"
                ns = tree_pool.tile([128, T * 32], bf16, tag=mtag, name=f"am{idx}_{k}")
                ns3 = ns[:].rearrange("p (t c) -> p t c", c=32)
                emit_tt(ns3, a16u, agg_parts[k], T, 32)
                a16u = ns3
            stage_state[idx] = (a16u, T, off)

        # tile index -> block index (blocks may have irregular sizes)
        blk_of_tile = {}
        for b, (t0b, ntb) in enumerate(blocks):
            for tt in range(t0b, t0b + ntb):
                blk_of_tile[tt] = b

        def stage2(idx):
            a16u, T, off = stage_state.pop(idx)
            # 32x32 stream transpose into the aggT blocks: the 32-wide unit is
            # (f, s1) pairs -> aggT rows 2f+s1, matching wet4's doubled rows.
            a16t = a16u.tensor
            t0 = off // 128
            t = 0
            while t < T:
                b = blk_of_tile[t0 + t]
                t0b, ntb = blocks[b]
                te = min(T, t0b + ntb - t0)
                span = te - t
                in_ap = bass.AP(a16t, t * 32,
                                [[T * 32, 128], [32, span], [1, 32]])
                with tc.high_priority(offset=PRIO_OFF):
                    nc.vector.transpose(
                        aggT_blocks[b][0:128,
                                       32 * (t0 + t - t0b):32 * (t0 + te - t0b)],
                        in_ap)
                t = te
                # a block whose last tile was just transposed can start its
                # ELU+MLP now; this class's later transposes aren't its deps
                done_rows = 128 * (t0 + t)
                if idx == len(kclasses) - 1 and t == T:
                    done_rows = R * 2
                while (next_block[0] < n_blocks
                       and done_rows >= 128 * (blocks[next_block[0]][0]
                                               + blocks[next_block[0]][1])):
                    nb = next_block[0]
                    emit_elu(nb)
                    if nb > 0:
                        emit_mlp(nb - 1)
                    next_block[0] += 1

        # Issue the GpSimd-offloaded classes' den/recip/alpha and ef fetch
        # EARLY (priority = issue order): GpSimd computes their prod+tree in
        # its idle mid-window, off the Vector engine's critical path.
        for gidx in range(n_cls):
            if on_gpsimd(gidx):
                prefetch_ef(gidx)
                prep(gidx)

        # V-tree classes: transpose right after own tree (in-order V queue
        # anyway). G-tree classes: lag one class so V-side stage1 work covers
        # the GpSimd tree latency.
        pending = None
        for idx in range(len(kclasses)):
            stage1(idx)
            if pending is not None:
                stage2(pending)
                pending = None
            if on_gpsimd(idx) and idx + 1 < len(kclasses):
                pending = idx
            else:
                stage2(idx)
        if pending is not None:
            stage2(pending)
        emit_mlp(n_blocks - 1)

    nc.compile()
    return nc


def kernel(**inputs):
    dst = np.asarray(inputs["dst"])
    plan = _build_plan(dst)
    in_maps = _shard_inputs(inputs, plan)
    nc = _build_kernel(plan)
    trace = bool(int(os.environ.get("GNN_PROFILE", "0")))
    if trace:
        try:
            _install_ntff_hook()
        except Exception:
            pass
    res = run_bass_kernel_spmd(nc, in_maps, core_ids=list(range(NCORES)),
                               trace=trace)
    kernel.last_results = res
    return _unshard(res.results, plan)


def _install_ntff_hook():
    """Recreate antenv.axon_hooks (absent in this image) so
    run_bass_kernel_spmd(trace=True) can NTFF-profile via libaxon_pjrt.so."""
    import contextlib, ctypes, sys, types
    if 'antenv.axon_hooks' in sys.modules:
        return
    lib = ctypes.CDLL('/opt/axon/libaxon_pjrt.so')
    lib.axon_start_nrt_profile.argtypes = [ctypes.POINTER(ctypes.c_int64), ctypes.c_size_t]
    lib.axon_start_nrt_profile.restype = ctypes.c_int64
    lib.axon_stop_nrt_profile.argtypes = [ctypes.c_char_p]
    lib.axon_stop_nrt_profile.restype = ctypes.c_int64

    @contextlib.contextmanager
    def _hook(output_dir, device_ids):
        import jax
        jax.devices()
        if device_ids:
            ids = (ctypes.c_int64 * len(device_ids))(*device_ids)
            rc = lib.axon_start_nrt_profile(ids, len(device_ids))
        else:
            rc = lib.axon_start_nrt_profile(None, 0)
        if rc != 0:
            raise RuntimeError(f"axon_start_nrt_profile rc={rc}")
        try:
            yield
        finally:
            n = lib.axon_stop_nrt_profile(str(output_dir).encode())
            print(f"ntff profile: {n} file(s) written to {output_dir}", file=sys.stderr)

    mod = types.ModuleType('antenv.axon_hooks')
    mod.get_axon_ntff_profile_hook = lambda: _hook
    mod.set_axon_ntff_profile_hook = lambda h: None
    import antenv
    antenv.axon_hooks = mod
    sys.modules['antenv.axon_hooks'] = mod


# revision 54
# speedup vs baseline: 1.1109x; 1.1109x over previous
"""AttentiveMLP GNN message-passing kernel for 8 Trainium2 NeuronCores.

Sharding: edges are partitioned BY DESTINATION NODE (each core owns ~N/8 nodes
plus all their incoming edges) so no cross-core collectives are needed. Nodes
are grouped on the host into degree-bucket classes (a pure layout/permutation
choice); within a class every node has exactly d edge slots (pad slots carry
logit -60 -> weight ~0), so segment softmax and the attention-weighted
aggregation are static dense ops over [128, T*d] tiles.

Key device-side structure (v2, tuned off the HW perfetto trace):
 - softmax max-subtraction is dropped (logits ~N(0,1): exp() cannot overflow,
   result is mathematically identical), killing two full edge passes.
 - alpha = exp(lg) * (1/den) is folded BEFORE the edge-feature multiply, so
   the aggregation tree output needs no post-normalization pass.
 - edge features ride in pair-major layout (t, s2, f, s1) with s1 a 2-slot
   pair: every level of the pairwise-halving reduction tree is a bf16
   tensor_tensor add over 32-element contiguous runs (the HW DVE only hits
   its 2x 16-bit mode on packed runs; the old feature-major layout decayed
   to 1x on deep levels), and the tree STOPS at slot-pairs: the 32x32 stream
   transpose lands pair partials on adjacent aggT rows (2f, 2f+1) and the
   ctx matmul's wet4 carries W_et on both rows, absorbing the final add into
   the PE contraction for free (the old layout wasted rows 16-31 on
   zero-weight duplicates).
 - ELU is computed as elu(x)+1 = min(exp(x),1) + relu(x) (two activations +
   one fused scalar_tensor_tensor); the +1 shift is folded into b1 on host.
 - logits ride bf16 as ONE merged [128, sum(T*d)] tensor (single descriptor
   gen, exps queue at the head of the Act engine); bf16 consts ride as one
   merged [128, 512] tensor.
 - classes are processed LARGEST FIRST so the final MLP blocks only wait on
   a tiny class's tree at the tail; the d=12 class tree rides GpSimd to
   offload the Vector engine.
 - MLP chunk g=3 feeds the PE from base partition 96 directly (legal when
   stationary and moving share the base), killing the cb3 copies.
 - scheduling: all exps are queued on the Act engine upfront; the class loop
   is software-pipelined (stage1 = den/recip/alpha/prod/tree of class i+1
   issues before stage2 = transpose of class i); each ready block's ELU head
   issues one block ahead of its MLP chunk loop; the last blocks alternate
   h/ov between Act and Vector (tensor_scalar add+max).
"""
import os
import numpy as np
import ml_dtypes
from contextlib import ExitStack

import concourse.bass as bass
import concourse.bacc as bacc
import concourse.tile as tile
import concourse.mybir as mybir
from concourse.bass_utils import run_bass_kernel_spmd

N_NODES = 100000
N_EDGES = 1600000
EF = 16
HID = 32
NF = 128
NCORES = 8
CHUNK = 512

f32 = mybir.dt.float32
bf16 = mybir.dt.bfloat16
BF = ml_dtypes.bfloat16

PAD_LG = -60.0  # exp(-60) ~ 8.8e-27: pad slots contribute ~0 weight, no inf/nan

BUCKETS = [8, 12, 14, 16, 18, 20, 24, 40,
           64, 128, 256, 1024, 4096, 65536, 1048576, 2097152]

# classes whose reduction tree runs on GpSimd (to offload Vector); these are
# scheduled FIRST so later V-side stage1 work covers the GpSimd latency
GP_TREE_D = ()
# number of trailing blocks whose h/ov relus alternate onto Vector
V_RELU_BLOCKS = 2
# priority offset for block-drain compute (transposes, ELU, MLP): the tile
# list scheduler otherwise prefers older stage1 work and delays the unlock
# of downstream engines
PRIO_OFF = 1_000_000


def _bin_blocks(n):
    """Contiguous ranges of power-of-2 width covering [0, n), descending."""
    out, s = [], 0
    for k in range(21, -1, -1):
        w = 1 << k
        if n & w:
            out.append((s, w))
            s += w
    return out


def _bucket_of(deg):
    b = np.zeros_like(deg)
    nz = deg > 0
    idx = np.searchsorted(np.asarray(BUCKETS), deg[nz])
    b[nz] = np.asarray(BUCKETS)[idx]
    return b


def _build_plan(dst):
    deg = np.bincount(dst, minlength=N_NODES)
    deg = _bucket_of(deg)
    order = np.argsort(deg, kind="stable")
    sdeg = deg[order]
    uniq, starts, counts = np.unique(sdeg, return_index=True, return_counts=True)
    ncls = len(uniq)
    rank = np.arange(N_NODES) - np.repeat(starts, counts)
    dev = rank % NCORES
    row_in_class = rank // NCORES
    n_pad = (counts + NCORES - 1) // NCORES
    n_pad = ((n_pad + 127) // 128) * 128   # 128-aligned class rows/offsets

    # Order: the tiniest class first (its ef lands within ~1us, so the whole
    # exp->den->alpha->prod->tree->transpose->ELU chain warms up the MLP
    # pipeline while the big ef transfers stream), then the cheapest class
    # that covers a full 16-tile block, then the rest LARGEST-work first so
    # the tail only waits on tiny classes.
    cls_ids = [ci for ci in range(ncls) if uniq[ci] > 0]
    cls_ids.sort(key=lambda ci: -int(uniq[ci]) * int(n_pad[ci]))
    head = []
    if cls_ids:
        tiny = cls_ids[-1]
        cls_ids.remove(tiny)
        head.append(tiny)
    starters = [ci for ci in cls_ids if n_pad[ci] >= 16 * 128]
    if starters:
        first = min(starters, key=lambda ci: int(uniq[ci]) * int(n_pad[ci]))
        cls_ids.remove(first)
        head.append(first)
    cls_ids = head + cls_ids
    if uniq[0] == 0:
        cls_ids = cls_ids + [0]
    offs_arr = np.zeros(ncls, dtype=np.int64)
    acc = 0
    for ci in cls_ids:
        offs_arr[ci] = acc
        acc += n_pad[ci]
    R = int(acc)

    cls_of_pos = np.repeat(np.arange(ncls), counts)
    lrow = offs_arr[cls_of_pos] + row_in_class

    node_dev = np.empty(N_NODES, dtype=np.int64)
    node_lrow = np.empty(N_NODES, dtype=np.int64)
    node_dev[order] = dev
    node_lrow[order] = lrow

    classes = [(int(uniq[ci]), int(n_pad[ci]), int(offs_arr[ci])) for ci in cls_ids]
    deg0_rows = classes[-1][1] if classes and classes[-1][0] == 0 else 0
    kclasses = [c for c in classes if c[0] > 0]
    zero_tail_start = R - deg0_rows

    n_tiles = R // 128
    # block b covers tiles [t0b, t0b+ntb); chunk g of block b is 32*ntb wide.
    # A small leading block matching the tiny first class lets the MLP
    # pipeline start while the big ef transfers are still streaming.
    blocks = []
    start = 0
    t_first = (kclasses[0][1] + 127) // 128 if kclasses else 0
    if 0 < t_first < 16:
        blocks.append((0, t_first))
        start = t_first
    while start + 16 <= n_tiles:
        blocks.append((start, 16))
        start += 16
    if start < n_tiles:
        blocks.append((start, n_tiles - start))

    return dict(
        uniq=uniq, counts=counts, node_dev=node_dev, node_lrow=node_lrow,
        R=R, n_tiles=n_tiles, blocks=blocks, kclasses=kclasses,
        zero_tail_start=zero_tail_start,
    )


def _mlpcol(r, blocks):
    """node row -> column in the transposed-MLP [NF, R] layout."""
    r = np.asarray(r)
    t = r // 128
    q = (r % 128) // 32
    c = r % 32
    t0s = np.array([t0b for (t0b, ntb) in blocks])
    nts = np.array([ntb for (t0b, ntb) in blocks])
    cbase = np.concatenate([[0], np.cumsum(4 * 32 * nts)])[:-1]
    b = np.searchsorted(t0s, t, side="right") - 1
    W = 32 * nts[b]
    return cbase[b] + q * W + 32 * (t - t0s[b]) + c


def _shard_inputs(inputs, plan):
    lg = np.ascontiguousarray(
        np.asarray(inputs["edge_logits"], dtype=np.float32).reshape(-1))
    ef = np.ascontiguousarray(np.asarray(inputs["edge_feats"], dtype=np.float32))
    nf = np.asarray(inputs["node_feats"], dtype=np.float32)
    dst = np.asarray(inputs["dst"])
    W_et = np.asarray(inputs["W_et"], dtype=np.float32)
    b_et = np.asarray(inputs["b_et"], dtype=np.float32)
    W1 = np.asarray(inputs["W1"], dtype=np.float32)
    b1 = np.asarray(inputs["b1"], dtype=np.float32)
    W2 = np.asarray(inputs["W2"], dtype=np.float32)
    b2 = np.asarray(inputs["b2"], dtype=np.float32)

    node_dev, node_lrow = plan["node_dev"], plan["node_lrow"]
    R, blocks = plan["R"], plan["blocks"]
    kclasses = plan["kclasses"]

    ekey = node_dev[dst] * R + node_lrow[dst]
    eorder = np.argsort(ekey, kind="stable")
    sk = ekey[eorder]
    newrun = np.empty(N_EDGES, dtype=bool)
    newrun[0] = True
    newrun[1:] = sk[1:] != sk[:-1]
    runstart = np.maximum.accumulate(np.where(newrun, np.arange(N_EDGES), 0))
    slot = np.arange(N_EDGES) - runstart
    e_dev = node_dev[dst[eorder]]
    e_lrow = node_lrow[dst[eorder]]
    lg_s = lg[eorder].astype(BF)
    ef_s = ef[eorder].astype(BF)

    lg_offs = []
    acc = 0
    for (d, npad, off) in kclasses:
        T = (npad + 127) // 128
        lg_offs.append(acc)
        acc += T * d
    lg_total = acc

    in_maps = [dict() for _ in range(NCORES)]
    for dv in range(NCORES):
        dmask = e_dev == dv
        d_lrow = e_lrow[dmask]
        d_slot = slot[dmask]
        d_lg = lg_s[dmask]
        d_ef = ef_s[dmask]
        lgall = np.full((128, lg_total), PAD_LG, dtype=BF)
        for idx, (d, npad, off) in enumerate(kclasses):
            T = (npad + 127) // 128
            cmask = (d_lrow >= off) & (d_lrow < off + npad)
            r = d_lrow[cmask] - off
            s = d_slot[cmask]
            p = r % 128
            t = r // 128
            # logits: [p, lg_offs + t*d + s] (slot s contiguous per tile)
            lgall[p, lg_offs[idx] + t * d + s] = d_lg[cmask]  # noqa (split below)
            # pair-major feature slots: s = 2*s2 + s1 ->
            #   [p, (t*(d//2) + s2)*2*EF + 2*f + s1]
            flat_ef = np.zeros((128, T * d * EF), dtype=BF)
            col = ((t * (d // 2) + s // 2) * 2 * EF + (s % 2))[:, None] \
                + np.arange(EF)[None, :] * 2
            flat_ef[p[:, None], col] = d_ef[cmask]
            in_maps[dv][f"ef{idx}"] = flat_ef
        # class-0 logits ride their own small DMA so the first exp can start
        # as early as possible; the rest follow in one transfer
        c0 = lg_offs[1] if len(kclasses) > 1 else lg_total
        in_maps[dv]["lg0"] = np.ascontiguousarray(lgall[:, :c0])
        in_maps[dv]["lgrest"] = np.ascontiguousarray(lgall[:, c0:])

    for dv in range(NCORES):
        sel = node_dev == dv
        nid = np.nonzero(sel)[0]
        lr = node_lrow[sel]
        nf_dev = np.zeros((R, NF), dtype=np.float32)
        nf_dev[_mlpcol(lr, blocks)] = nf[nid]
        in_maps[dv]["nfT"] = np.ascontiguousarray(nf_dev.T).astype(BF)

    # device computes cb' = elu(ctx)+1 = min(exp(z),1)+relu(z); fold the -1
    # correction into b1: h = relu(W1c^T cb' + W1n^T nf + (b1 - colsum(W1c)))
    b1 = b1 - W1[:HID].sum(axis=0)
    # wet4 rows carry W_et on BOTH pair rows (2f, 2f+1): the ctx matmul sums
    # the two slot-pair partials the transpose lands on adjacent rows.
    wet4 = np.zeros((128, 128), dtype=BF)
    bet4 = np.zeros((128, 1), dtype=np.float32)
    for g in range(4):
        for s1 in range(2):
            wet4[32 * g + s1:32 * g + 2 * EF + s1:2, 32 * g:32 * g + HID] = \
                W_et.astype(BF)
        bet4[32 * g:32 * g + HID, 0] = b_et
    cb16 = np.zeros((128, 512), dtype=BF)
    cb16[:, 0:128] = wet4
    cb16[:, 128:256] = np.tile(W1[:HID], (4, 1)).astype(BF)
    cb16[:, 256:384] = W1[HID:].astype(BF)
    cb16[:, 384:512] = W2.astype(BF)
    cf32 = np.zeros((128, 3), dtype=np.float32)
    cf32[:, 0:1] = bet4
    cf32[:, 1:2] = b1.reshape(NF, 1)
    cf32[:, 2:3] = b2.reshape(NF, 1)
    for dv in range(NCORES):
        in_maps[dv]["cb16"] = cb16.copy()
        in_maps[dv]["cf32"] = cf32.copy()
    return in_maps


def _unshard(results, plan):
    node_dev, node_lrow = plan["node_dev"], plan["node_lrow"]
    blocks = plan["blocks"]
    out = np.empty((N_NODES, NF), dtype=np.float32)
    for dv in range(NCORES):
        sel = node_dev == dv
        nid = np.nonzero(sel)[0]
        lr = node_lrow[sel]
        out_dev = results[dv]["outT"].T.astype(np.float32)
        out[nid] = out_dev[_mlpcol(lr, blocks)]
    return out


def _build_kernel(plan):
    kclasses = plan["kclasses"]
    R = plan["R"]
    blocks = plan["blocks"]
    n_blocks = len(blocks)
    zts = plan["zero_tail_start"]

    nc = bacc.Bacc("TRN2", target_bir_lowering=False, debug=False,
                   num_devices=NCORES)

    lg_offs = []
    acc = 0
    for (d, npad, off) in kclasses:
        T = (npad + 127) // 128
        lg_offs.append(acc)
        acc += T * d
    lg_total = acc

    ef_d = []
    for idx, (d, npad, off) in enumerate(kclasses):
        T = (npad + 127) // 128
        ef_d.append(nc.dram_tensor(f"ef{idx}", [128, T * d * EF], bf16,
                                   kind="ExternalInput"))
    c0 = lg_offs[1] if len(kclasses) > 1 else lg_total
    lg0_d = nc.dram_tensor("lg0", [128, c0], bf16, kind="ExternalInput")
    lgrest_d = nc.dram_tensor("lgrest", [128, lg_total - c0], bf16,
                              kind="ExternalInput")
    nfT_d = nc.dram_tensor("nfT", [NF, R], bf16, kind="ExternalInput")
    cb16_d = nc.dram_tensor("cb16", [128, 512], bf16, kind="ExternalInput")
    cf32_d = nc.dram_tensor("cf32", [128, 3], f32, kind="ExternalInput")
    out_d = nc.dram_tensor("outT", [NF, R], bf16, kind="ExternalOutput")

    # column base of each block in the [NF, R] layout
    col_base = []
    acc = 0
    for (t0b, ntb) in blocks:
        col_base.append(acc)
        acc += 4 * 32 * ntb
    assert acc == R

    with tile.TileContext(nc) as tc, ExitStack() as ctx:
        const_pool = ctx.enter_context(tc.tile_pool(name="const", bufs=1))
        agg_pool = ctx.enter_context(tc.tile_pool(name="agg", bufs=1))
        cls_pool = ctx.enter_context(tc.tile_pool(name="cls", bufs=4))
        work_pool = ctx.enter_context(tc.tile_pool(name="work", bufs=3))
        tree_pool = ctx.enter_context(tc.tile_pool(name="tree", bufs=3))
        small_pool = ctx.enter_context(tc.tile_pool(name="small", bufs=4))
        mlp_pool = ctx.enter_context(tc.tile_pool(name="mlp", bufs=3))
        ctx_pool = ctx.enter_context(tc.tile_pool(name="ctxs", bufs=2))
        ctx_psum = ctx.enter_context(tc.tile_pool(name="ctxp", bufs=2, space="PSUM"))
        mlp1_psum = ctx.enter_context(tc.tile_pool(name="m1p", bufs=4, space="PSUM"))
        mlp2_psum = ctx.enter_context(tc.tile_pool(name="m2p", bufs=2, space="PSUM"))

        # Two logits DMAs (class 0's small slice lands first so exp0 starts
        # ASAP) and all exps queued at the head of the Act engine's in-order
        # queue.
        lgt = cls_pool.tile([128, lg_total], bf16, name="lgall")
        nc.sync.dma_start(lgt[0:128, 0:c0], lg0_d.ap())
        nc.sync.dma_start(lgt[0:128, c0:lg_total], lgrest_d.ap())
        x_tiles = []
        for ci, (dc, npadc, _) in enumerate(kclasses):
            Tc = (npadc + 127) // 128
            xt = cls_pool.tile([128, Tc * dc], bf16, tag=f"x{ci}", name=f"x{ci}")
            nc.scalar.activation(xt[:], lgt[0:128, lg_offs[ci]:lg_offs[ci] + Tc * dc],
                                 mybir.ActivationFunctionType.Exp)
            x_tiles.append(xt)
        prefetched = {}

        def prefetch_ef(pi):
            if pi >= len(kclasses) or pi in prefetched:
                return
            dp, npadp, _ = kclasses[pi]
            Tp = (npadp + 127) // 128
            eftp = cls_pool.tile([128, Tp * dp * EF], bf16, tag="ef", name=f"eft{pi}")
            nc.sync.dma_start(eftp[:], ef_d[pi].ap())
            prefetched[pi] = eftp

        prefetch_ef(0)
        prefetch_ef(1)

        consts = const_pool.tile([128, 512], bf16, name="cb16c")
        nc.gpsimd.dma_start(consts[:], cb16_d.ap())
        constf = const_pool.tile([128, 3], f32, name="cf32c")
        nc.gpsimd.dma_start(constf[:], cf32_d.ap())
        wet4 = consts[0:128, 0:128]
        w1c = consts[0:128, 128:256]
        w1n = consts[0:128, 256:384]
        w2 = consts[0:128, 384:512]
        bet4 = constf[0:128, 0:1]
        b1 = constf[0:128, 1:2]
        b2 = constf[0:128, 2:3]

        aggT_blocks = []
        for b, (t0b, ntb) in enumerate(blocks):
            ab = agg_pool.tile([128, 32 * ntb], bf16, name=f"aggT{b}")
            aggT_blocks.append(ab)
        # deg0 tail rows (none for this input, but keep correct in general):
        # their aggT columns are never written by any class; zero them so the
        # ctx matmul sees context 0 (b_et==0 -> elu path produces cb'=1).
        if zts < R:
            t_lo = zts // 128
            for b, (t0b, ntb) in enumerate(blocks):
                lo = max(t0b, t_lo)
                hi = t0b + ntb
                if lo < hi:
                    nc.gpsimd.memset(
                        aggT_blocks[b][0:128, 32 * (lo - t0b):32 * (hi - t0b)], 0.0)

        # ---------------- Phase B+C: ELU + MLP per block ----------------
        nf_tiles = {}

        def prefetch_nf(b):
            if b >= n_blocks or b in nf_tiles:
                return
            t0b, ntb = blocks[b]
            W = 32 * ntb
            cb4 = col_base[b]
            t = mlp_pool.tile([NF, 4 * W], bf16, tag="nfblk", name=f"nfblk{b}")
            nc.sync.dma_start(t[:], nfT_d.ap()[:, cb4:cb4 + 4 * W])
            nf_tiles[b] = t

        cb_tiles = {}

        def emit_elu(b):
            t0b, ntb = blocks[b]
            W = 32 * ntb
            with tc.high_priority(offset=PRIO_OFF):
                ctx4 = ctx_psum.tile([128, W], f32, tag="ctx4", name=f"ctx4_{b}")
                nc.tensor.matmul(ctx4[:], wet4, aggT_blocks[b][:])
                s1 = mlp_pool.tile([128, W], bf16, tag="s1", name=f"s1_{b}")
                if b >= n_blocks - V_RELU_BLOCKS:
                    nc.vector.tensor_scalar(s1[:], ctx4[:], bet4, 0.0,
                                            mybir.AluOpType.add,
                                            mybir.AluOpType.max)
                else:
                    nc.scalar.activation(s1[:], ctx4[:],
                                         mybir.ActivationFunctionType.Relu,
                                         bias=bet4, scale=1.0)
                s2 = mlp_pool.tile([128, W], bf16, tag="s2", name=f"s2_{b}")
                nc.scalar.activation(s2[:], ctx4[:],
                                     mybir.ActivationFunctionType.Exp,
                                     bias=bet4, scale=1.0)
            # cb' = elu(ctx)+1 = min(exp(z),1) + relu(z); the +1 shift is
            # compensated in b1 on the host. s1/s2 ride bf16 so the stst
            # qualifies for the DVE's 2x 16-bit mode.
            cb = ctx_pool.tile([128, W], bf16, tag="cb", name=f"cb_{b}")
            with tc.high_priority(offset=PRIO_OFF):
                nc.vector.scalar_tensor_tensor(cb[:], s2[:], 1.0, s1[:],
                                               mybir.AluOpType.min,
                                               mybir.AluOpType.add)
            cb_tiles[b] = cb
            prefetch_nf(b)
            prefetch_nf(b + 1)

        def emit_mlp(b):
            t0b, ntb = blocks[b]
            W = 32 * ntb
            cb4 = col_base[b]
            cb = cb_tiles.pop(b)
            nfblk = nf_tiles.pop(b)
            oblk = mlp_pool.tile([NF, 4 * W], bf16, tag="oblk", name=f"oblk{b}")
            last = b == n_blocks - 1
            with tc.high_priority(offset=PRIO_OFF):
                for g in range(4):
                    j = 4 * b + g
                    nfb = nfblk[0:NF, g * W:(g + 1) * W]
                    ps1 = mlp1_psum.tile([NF, W], f32, tag="ps1", name=f"ps1_{j}")
                    cbg = cb[32 * g:32 * (g + 1), :]
                    w1cg = w1c[32 * g:32 * (g + 1), 0:NF]
                    nc.tensor.matmul(ps1[:], w1n, nfb, start=True, stop=False)
                    # base partition 96 is legal for a 32-row tile but the
                    # default inference path refuses it; pass it explicitly
                    nc.tensor.matmul(ps1[:], w1cg, cbg, start=False, stop=True,
                                     tile_position=(32 * g, 0))
                    h = mlp_pool.tile([NF, W], bf16, tag="h", name=f"h{j}")
                    tail = b >= n_blocks - V_RELU_BLOCKS
                    if tail and j % 2 == 1:
                        nc.vector.tensor_scalar(h[:], ps1[:], b1, 0.0,
                                                mybir.AluOpType.add,
                                                mybir.AluOpType.max)
                    else:
                        nc.scalar.activation(h[:], ps1[:],
                                             mybir.ActivationFunctionType.Relu,
                                             bias=b1, scale=1.0)
                    ps2 = mlp2_psum.tile([NF, W], f32, tag="ps2", name=f"ps2_{j}")
                    nc.tensor.matmul(ps2[:], w2, h[:])
                    ov = oblk[0:NF, g * W:(g + 1) * W]
                    if tail and j % 2 == 0:
                        nc.vector.tensor_scalar(ov, ps2[:], b2, 0.0,
                                                mybir.AluOpType.add,
                                                mybir.AluOpType.max)
                    else:
                        nc.scalar.activation(ov, ps2[:],
                                             mybir.ActivationFunctionType.Relu,
                                             bias=b2, scale=1.0)
            nc.gpsimd.dma_start(out_d.ap()[:, cb4:cb4 + 2 * W],
                                oblk[0:NF, 0:2 * W])
            nc.gpsimd.dma_start(out_d.ap()[:, cb4 + 2 * W:cb4 + 4 * W],
                                oblk[0:NF, 2 * W:4 * W])

        next_block = [0]

        # ------------- Phase A: per-class segment softmax + aggregation ------
        # Two-stage software pipeline: stage1 (den/recip/alpha/prod/tree) of
        # class i+1 is ISSUED before stage2 (transpose/emit) of class i, so
        # the in-order Vector queue always holds independent work ahead of
        # any op that waits on a GpSimd tree.
        stage_state = {}

        # The LAST class's prod+tree ride GpSimd (chunked so the
        # high-priority alphas still preempt): its den is hoisted early by
        # the scheduler, GpSimd computes it during its idle mid-window, and
        # the Vector engine reaches the final transposes sooner.
        n_cls = len(kclasses)

        def on_gpsimd(idx):
            # measured: offloading even the smallest tail class to GpSimd
            # shifts the ef DMA order and the V tail right; net negative
            return False

        alphas = {}

        def prep(idx):
            """den -> recip -> alpha for one class (idempotent)."""
            if idx in alphas:
                return alphas[idx]
            d, npad, off = kclasses[idx]
            T = (npad + 127) // 128
            xt = x_tiles[idx]
            x3 = xt[:].rearrange("p (t d) -> p t d", t=T)
            # den[p,t] = sum_d x; alpha = x * (1/den) pre-normalizes so the
            # tree output feeds the transpose directly. alpha rides the
            # otherwise-idle GpSimd (SBUF-only op); recip is DVE-only.
            den = small_pool.tile([128, T], f32, tag="den", name=f"den{idx}")
            nc.vector.tensor_reduce(den[:], x3, mybir.AxisListType.X,
                                    mybir.AluOpType.add)
            rd = small_pool.tile([128, T], f32, tag="rd", name=f"rd{idx}")
            nc.vector.reciprocal(rd[:], den[:])
            atag = f"gal{idx}" if on_gpsimd(idx) else "al"
            alpha = small_pool.tile([128, T * d], bf16, tag=atag, name=f"al{idx}")
            alpha3 = alpha[:].rearrange("p (t d) -> p t d", t=T)
            rd_b = rd[:].unsqueeze(2).broadcast_to([128, T, d])
            # class 0 is the latency-critical warmup chain: keep its alpha on
            # the DVE (no cross-engine hop); later classes ride GpSimd at
            # high priority so queued cb work never delays the prod chain
            if idx == 0:
                nc.vector.tensor_tensor(alpha3, x3, rd_b, mybir.AluOpType.mult)
            else:
                with tc.high_priority(offset=PRIO_OFF):
                    nc.gpsimd.tensor_tensor(alpha3, x3, rd_b,
                                            mybir.AluOpType.mult)
            alphas[idx] = alpha
            return alpha

        def stage1(idx):
            d, npad, off = kclasses[idx]
            d2 = d // 2
            T = (npad + 127) // 128
            prefetch_ef(idx + 2)
            eft = prefetched[idx]
            alpha = prep(idx)

            # prod[p, ts2, f, s1] = ef * alpha  (bf16 2x mode: packed pairs).
            # A small leading tile-slice rides GpSimd (slow but idle) so the
            # Vector engine finishes stage1 sooner.
            ptag = f"gprod{idx}" if on_gpsimd(idx) else "prod"
            prod = work_pool.tile([128, T * d * EF], bf16, tag=ptag, name=f"prod{idx}")
            ef4 = eft[:].rearrange("p (ts f s1) -> p ts f s1", f=EF, s1=2)
            al4 = alpha[:].rearrange("p (ts s1) -> p ts s1", s1=2) \
                .unsqueeze(2).broadcast_to([128, T * d2, EF, 2])
            prod4 = prod[:].rearrange("p (ts f s1) -> p ts f s1", f=EF, s1=2)
            gp = on_gpsimd(idx)

            def emit_tt(out_ap, a_ap, b_ap, n1, per_unit_free,
                        op=mybir.AluOpType.add):
                """tensor_tensor on V, or on G chunked along dim 1 so
                high-priority alphas can preempt between pieces."""
                if not gp:
                    nc.vector.tensor_tensor(out_ap, a_ap, b_ap, op)
                    return
                gsz = max(1, 640 // per_unit_free)
                nd = len(out_ap.shape)
                s = 0
                while s < n1:
                    e = min(n1, s + gsz)
                    if nd == 4:
                        sl = (slice(None), slice(s, e), slice(None), slice(None))
                    else:
                        sl = (slice(None), slice(s, e), slice(None))
                    nc.gpsimd.tensor_tensor(out_ap[sl], a_ap[sl], b_ap[sl], op)
                    s = e

            emit_tt(prod4, al4, ef4, T * d2, 32, mybir.AluOpType.mult)

            # pairwise-halving tree over s2 (32-elem contiguous runs at every
            # level); STOPS at slot-pairs -- the ctx matmul absorbs the last
            # add via the doubled wet4 rows.
            prodc = prod[:].rearrange("p (t s c) -> p t s c", t=T, c=32)
            agg_parts = []
            for bi, (s0, w) in enumerate(_bin_blocks(d2)):
                cur = prodc[:, :, s0:s0 + w, :]
                cw = w
                while cw > 1:
                    half = cw // 2
                    ttag = (f"gag{idx}_{bi}_{half}" if on_gpsimd(idx)
                            else f"ag{bi}_{half}")
                    nt = tree_pool.tile([128, T * half * 32], bf16,
                                        tag=ttag, name=f"ag{idx}_{bi}_{half}")
                    nt4 = nt[:].rearrange("p (t s c) -> p t s c", t=T, c=32)
                    emit_tt(nt4, cur[:, :, 0:half, :], cur[:, :, half:cw, :],
                            T, 32 * half)
                    cur = nt4
                    cw = half
                agg_parts.append(cur.rearrange("p t s c -> p t (s c)"))
            a16u = agg_parts[0]
            for k in range(1, len(agg_parts)):
                mtag = f"gam{idx}_{k}" if on_gpsimd(idx) else f"am{k}"
                ns = tree_pool.tile([128, T * 32], bf16, tag=mtag, name=f"am{idx}_{k}")
                ns3 = ns[:].rearrange("p (t c) -> p t c", c=32)
                emit_tt(ns3, a16u, agg_parts[k], T, 32)
                a16u = ns3
            stage_state[idx] = (a16u, T, off)

        # tile index -> block index (blocks may have irregular sizes)
        blk_of_tile = {}
        for b, (t0b, ntb) in enumerate(blocks):
            for tt in range(t0b, t0b + ntb):
                blk_of_tile[tt] = b

        def stage2(idx):
            a16u, T, off = stage_state.pop(idx)
            # 32x32 stream transpose into the aggT blocks: the 32-wide unit is
            # (f, s1) pairs -> aggT rows 2f+s1, matching wet4's doubled rows.
            a16t = a16u.tensor
            t0 = off // 128
            t = 0
            while t < T:
                b = blk_of_tile[t0 + t]
                t0b, ntb = blocks[b]
                te = min(T, t0b + ntb - t0)
                span = te - t
                in_ap = bass.AP(a16t, t * 32,
                                [[T * 32, 128], [32, span], [1, 32]])
                with tc.high_priority(offset=PRIO_OFF):
                    nc.vector.transpose(
                        aggT_blocks[b][0:128,
                                       32 * (t0 + t - t0b):32 * (t0 + te - t0b)],
                        in_ap)
                t = te
                # a block whose last tile was just transposed can start its
                # ELU+MLP now; this class's later transposes aren't its deps
                done_rows = 128 * (t0 + t)
                if idx == len(kclasses) - 1 and t == T:
                    done_rows = R * 2
                while (next_block[0] < n_blocks
                       and done_rows >= 128 * (blocks[next_block[0]][0]
                                               + blocks[next_block[0]][1])):
                    nb = next_block[0]
                    emit_elu(nb)
                    if nb > 0:
                        emit_mlp(nb - 1)
                    next_block[0] += 1

        # Issue the GpSimd-offloaded classes' den/recip/alpha and ef fetch
        # EARLY (priority = issue order): GpSimd computes their prod+tree in
        # its idle mid-window, off the Vector engine's critical path.
        for gidx in range(n_cls):
            if on_gpsimd(gidx):
                prefetch_ef(gidx)
                prep(gidx)

        # V-tree classes: transpose right after own tree (in-order V queue
        # anyway). G-tree classes: lag one class so V-side stage1 work covers
        # the GpSimd tree latency.
        pending = None
        for idx in range(len(kclasses)):
            stage1(idx)
            if pending is not None:
                stage2(pending)
                pending = None
            if on_gpsimd(idx) and idx + 1 < len(kclasses):
                pending = idx
            else:
                stage2(idx)
        if pending is not None:
            stage2(pending)
        emit_mlp(n_blocks - 1)

    nc.compile()
    return nc


def kernel(**inputs):
    dst = np.asarray(inputs["dst"])
    plan = _build_plan(dst)
    in_maps = _shard_inputs(inputs, plan)
    nc = _build_kernel(plan)
    trace = bool(int(os.environ.get("GNN_PROFILE", "0")))
    if trace:
        try:
            _install_ntff_hook()
        except Exception:
            pass
    res = run_bass_kernel_spmd(nc, in_maps, core_ids=list(range(NCORES)),
                               trace=trace)
    kernel.last_results = res
    return _unshard(res.results, plan)


def _install_ntff_hook():
    """Recreate antenv.axon_hooks (absent in this image) so
    run_bass_kernel_spmd(trace=True) can NTFF-profile via libaxon_pjrt.so."""
    import contextlib, ctypes, sys, types
    if 'antenv.axon_hooks' in sys.modules:
        return
    lib = ctypes.CDLL('/opt/axon/libaxon_pjrt.so')
    lib.axon_start_nrt_profile.argtypes = [ctypes.POINTER(ctypes.c_int64), ctypes.c_size_t]
    lib.axon_start_nrt_profile.restype = ctypes.c_int64
    lib.axon_stop_nrt_profile.argtypes = [ctypes.c_char_p]
    lib.axon_stop_nrt_profile.restype = ctypes.c_int64

    @contextlib.contextmanager
    def _hook(output_dir, device_ids):
        import jax
        jax.devices()
        if device_ids:
            ids = (ctypes.c_int64 * len(device_ids))(*device_ids)
            rc = lib.axon_start_nrt_profile(ids, len(device_ids))
        else:
            rc = lib.axon_start_nrt_profile(None, 0)
        if rc != 0:
            raise RuntimeError(f"axon_start_nrt_profile rc={rc}")
        try:
            yield
        finally:
            n = lib.axon_stop_nrt_profile(str(output_dir).encode())
            print(f"ntff profile: {n} file(s) written to {output_dir}", file=sys.stderr)

    mod = types.ModuleType('antenv.axon_hooks')
    mod.get_axon_ntff_profile_hook = lambda: _hook
    mod.set_axon_ntff_profile_hook = lambda h: None
    import antenv
    antenv.axon_hooks = mod
    sys.modules['antenv.axon_hooks'] = mod


# revision 55
# speedup vs baseline: 1.1267x; 1.0142x over previous
"""AttentiveMLP GNN message-passing kernel for 8 Trainium2 NeuronCores.

Sharding: edges are partitioned BY DESTINATION NODE (each core owns ~N/8 nodes
plus all their incoming edges) so no cross-core collectives are needed. Nodes
are grouped on the host into degree-bucket classes (a pure layout/permutation
choice); within a class every node has exactly d edge slots (pad slots carry
logit -60 -> weight ~0), so segment softmax and the attention-weighted
aggregation are static dense ops over [128, T*d] tiles.

Key device-side structure (v2, tuned off the HW perfetto trace):
 - softmax max-subtraction is dropped (logits ~N(0,1): exp() cannot overflow,
   result is mathematically identical), killing two full edge passes.
 - alpha = exp(lg) * (1/den) is folded BEFORE the edge-feature multiply, so
   the aggregation tree output needs no post-normalization pass.
 - edge features ride in pair-major layout (t, s2, f, s1) with s1 a 2-slot
   pair: every level of the pairwise-halving reduction tree is a bf16
   tensor_tensor add over 32-element contiguous runs (the HW DVE only hits
   its 2x 16-bit mode on packed runs; the old feature-major layout decayed
   to 1x on deep levels), and the tree STOPS at slot-pairs: the 32x32 stream
   transpose lands pair partials on adjacent aggT rows (2f, 2f+1) and the
   ctx matmul's wet4 carries W_et on both rows, absorbing the final add into
   the PE contraction for free (the old layout wasted rows 16-31 on
   zero-weight duplicates).
 - ELU is computed as elu(x)+1 = min(exp(x),1) + relu(x) (two activations +
   one fused scalar_tensor_tensor); the +1 shift is folded into b1 on host.
 - logits ride bf16 as ONE merged [128, sum(T*d)] tensor (single descriptor
   gen, exps queue at the head of the Act engine); bf16 consts ride as one
   merged [128, 512] tensor.
 - classes are processed LARGEST FIRST so the final MLP blocks only wait on
   a tiny class's tree at the tail; the d=12 class tree rides GpSimd to
   offload the Vector engine.
 - MLP chunk g=3 feeds the PE from base partition 96 directly (legal when
   stationary and moving share the base), killing the cb3 copies.
 - scheduling: all exps are queued on the Act engine upfront; the class loop
   is software-pipelined (stage1 = den/recip/alpha/prod/tree of class i+1
   issues before stage2 = transpose of class i); each ready block's ELU head
   issues one block ahead of its MLP chunk loop; the last blocks alternate
   h/ov between Act and Vector (tensor_scalar add+max).
"""
import os
import numpy as np
import ml_dtypes
from contextlib import ExitStack

import concourse.bass as bass
import concourse.bacc as bacc
import concourse.tile as tile
import concourse.mybir as mybir
from concourse.bass_utils import run_bass_kernel_spmd

N_NODES = 100000
N_EDGES = 1600000
EF = 16
HID = 32
NF = 128
NCORES = 8
CHUNK = 512

f32 = mybir.dt.float32
bf16 = mybir.dt.bfloat16
BF = ml_dtypes.bfloat16

PAD_LG = -60.0  # exp(-60) ~ 8.8e-27: pad slots contribute ~0 weight, no inf/nan

BUCKETS = [8, 12, 14, 16, 18, 20, 24, 40,
           64, 128, 256, 1024, 4096, 65536, 1048576, 2097152]

# classes whose reduction tree runs on GpSimd (to offload Vector); these are
# scheduled FIRST so later V-side stage1 work covers the GpSimd latency
GP_TREE_D = ()
# number of trailing blocks whose h/ov relus alternate onto Vector
V_RELU_BLOCKS = 2
# priority offset for block-drain compute (transposes, ELU, MLP): the tile
# list scheduler otherwise prefers older stage1 work and delays the unlock
# of downstream engines
PRIO_OFF = 1_000_000


def _bin_blocks(n):
    """Contiguous ranges of power-of-2 width covering [0, n), descending."""
    out, s = [], 0
    for k in range(21, -1, -1):
        w = 1 << k
        if n & w:
            out.append((s, w))
            s += w
    return out


def _bucket_of(deg):
    b = np.zeros_like(deg)
    nz = deg > 0
    idx = np.searchsorted(np.asarray(BUCKETS), deg[nz])
    b[nz] = np.asarray(BUCKETS)[idx]
    return b


def _build_plan(dst):
    deg = np.bincount(dst, minlength=N_NODES)
    deg = _bucket_of(deg)
    order = np.argsort(deg, kind="stable")
    sdeg = deg[order]
    uniq, starts, counts = np.unique(sdeg, return_index=True, return_counts=True)
    ncls = len(uniq)
    rank = np.arange(N_NODES) - np.repeat(starts, counts)
    dev = rank % NCORES
    row_in_class = rank // NCORES
    n_pad = (counts + NCORES - 1) // NCORES
    n_pad = ((n_pad + 127) // 128) * 128   # 128-aligned class rows/offsets

    # Order: the tiniest class first (its ef lands within ~1us, so the whole
    # exp->den->alpha->prod->tree->transpose->ELU chain warms up the MLP
    # pipeline while the big ef transfers stream), then the cheapest class
    # that covers a full 16-tile block, then the rest LARGEST-work first so
    # the tail only waits on tiny classes.
    cls_ids = [ci for ci in range(ncls) if uniq[ci] > 0]
    cls_ids.sort(key=lambda ci: -int(uniq[ci]) * int(n_pad[ci]))
    head = []
    if cls_ids:
        tiny = cls_ids[-1]
        cls_ids.remove(tiny)
        head.append(tiny)
    starters = [ci for ci in cls_ids if n_pad[ci] >= 16 * 128]
    if starters:
        first = min(starters, key=lambda ci: int(uniq[ci]) * int(n_pad[ci]))
        cls_ids.remove(first)
        head.append(first)
    cls_ids = head + cls_ids
    if uniq[0] == 0:
        cls_ids = cls_ids + [0]
    offs_arr = np.zeros(ncls, dtype=np.int64)
    acc = 0
    for ci in cls_ids:
        offs_arr[ci] = acc
        acc += n_pad[ci]
    R = int(acc)

    cls_of_pos = np.repeat(np.arange(ncls), counts)
    lrow = offs_arr[cls_of_pos] + row_in_class

    node_dev = np.empty(N_NODES, dtype=np.int64)
    node_lrow = np.empty(N_NODES, dtype=np.int64)
    node_dev[order] = dev
    node_lrow[order] = lrow

    classes = [(int(uniq[ci]), int(n_pad[ci]), int(offs_arr[ci])) for ci in cls_ids]
    deg0_rows = classes[-1][1] if classes and classes[-1][0] == 0 else 0
    kclasses = [c for c in classes if c[0] > 0]
    zero_tail_start = R - deg0_rows

    n_tiles = R // 128
    # block b covers tiles [t0b, t0b+ntb); chunk g of block b is 32*ntb wide.
    # A small leading block matching the tiny first class lets the MLP
    # pipeline start while the big ef transfers are still streaming.
    blocks = []
    start = 0
    t_first = (kclasses[0][1] + 127) // 128 if kclasses else 0
    if 0 < t_first < 16:
        blocks.append((0, t_first))
        start = t_first
    while start + 16 <= n_tiles:
        blocks.append((start, 16))
        start += 16
    if start < n_tiles:
        blocks.append((start, n_tiles - start))

    return dict(
        uniq=uniq, counts=counts, node_dev=node_dev, node_lrow=node_lrow,
        R=R, n_tiles=n_tiles, blocks=blocks, kclasses=kclasses,
        zero_tail_start=zero_tail_start,
    )


def _mlpcol(r, blocks):
    """node row -> column in the transposed-MLP [NF, R] layout."""
    r = np.asarray(r)
    t = r // 128
    q = (r % 128) // 32
    c = r % 32
    t0s = np.array([t0b for (t0b, ntb) in blocks])
    nts = np.array([ntb for (t0b, ntb) in blocks])
    cbase = np.concatenate([[0], np.cumsum(4 * 32 * nts)])[:-1]
    b = np.searchsorted(t0s, t, side="right") - 1
    W = 32 * nts[b]
    return cbase[b] + q * W + 32 * (t - t0s[b]) + c


def _shard_inputs(inputs, plan):
    lg = np.ascontiguousarray(
        np.asarray(inputs["edge_logits"], dtype=np.float32).reshape(-1))
    ef = np.ascontiguousarray(np.asarray(inputs["edge_feats"], dtype=np.float32))
    nf = np.asarray(inputs["node_feats"], dtype=np.float32)
    dst = np.asarray(inputs["dst"])
    W_et = np.asarray(inputs["W_et"], dtype=np.float32)
    b_et = np.asarray(inputs["b_et"], dtype=np.float32)
    W1 = np.asarray(inputs["W1"], dtype=np.float32)
    b1 = np.asarray(inputs["b1"], dtype=np.float32)
    W2 = np.asarray(inputs["W2"], dtype=np.float32)
    b2 = np.asarray(inputs["b2"], dtype=np.float32)

    node_dev, node_lrow = plan["node_dev"], plan["node_lrow"]
    R, blocks = plan["R"], plan["blocks"]
    kclasses = plan["kclasses"]

    ekey = node_dev[dst] * R + node_lrow[dst]
    eorder = np.argsort(ekey, kind="stable")
    sk = ekey[eorder]
    newrun = np.empty(N_EDGES, dtype=bool)
    newrun[0] = True
    newrun[1:] = sk[1:] != sk[:-1]
    runstart = np.maximum.accumulate(np.where(newrun, np.arange(N_EDGES), 0))
    slot = np.arange(N_EDGES) - runstart
    e_dev = node_dev[dst[eorder]]
    e_lrow = node_lrow[dst[eorder]]
    lg_s = lg[eorder].astype(BF)
    ef_s = ef[eorder].astype(BF)

    lg_offs = []
    acc = 0
    for (d, npad, off) in kclasses:
        T = (npad + 127) // 128
        lg_offs.append(acc)
        acc += T * d
    lg_total = acc

    in_maps = [dict() for _ in range(NCORES)]
    for dv in range(NCORES):
        dmask = e_dev == dv
        d_lrow = e_lrow[dmask]
        d_slot = slot[dmask]
        d_lg = lg_s[dmask]
        d_ef = ef_s[dmask]
        lgall = np.full((128, lg_total), PAD_LG, dtype=BF)
        for idx, (d, npad, off) in enumerate(kclasses):
            T = (npad + 127) // 128
            cmask = (d_lrow >= off) & (d_lrow < off + npad)
            r = d_lrow[cmask] - off
            s = d_slot[cmask]
            p = r % 128
            t = r // 128
            # logits: [p, lg_offs + t*d + s] (slot s contiguous per tile)
            lgall[p, lg_offs[idx] + t * d + s] = d_lg[cmask]  # noqa (split below)
            # pair-major feature slots: s = 2*s2 + s1 ->
            #   [p, (t*(d//2) + s2)*2*EF + 2*f + s1]
            flat_ef = np.zeros((128, T * d * EF), dtype=BF)
            col = ((t * (d // 2) + s // 2) * 2 * EF + (s % 2))[:, None] \
                + np.arange(EF)[None, :] * 2
            flat_ef[p[:, None], col] = d_ef[cmask]
            in_maps[dv][f"ef{idx}"] = flat_ef
        # class-0 logits ride their own small DMA so the first exp can start
        # as early as possible; the rest follow in one transfer
        c0 = lg_offs[1] if len(kclasses) > 1 else lg_total
        in_maps[dv]["lg0"] = np.ascontiguousarray(lgall[:, :c0])
        in_maps[dv]["lgrest"] = np.ascontiguousarray(lgall[:, c0:])

    for dv in range(NCORES):
        sel = node_dev == dv
        nid = np.nonzero(sel)[0]
        lr = node_lrow[sel]
        nf_dev = np.zeros((R, NF), dtype=np.float32)
        nf_dev[_mlpcol(lr, blocks)] = nf[nid]
        in_maps[dv]["nfT"] = np.ascontiguousarray(nf_dev.T).astype(BF)

    # device computes cb' = elu(ctx)+1 = min(exp(z),1)+relu(z); fold the -1
    # correction into b1: h = relu(W1c^T cb' + W1n^T nf + (b1 - colsum(W1c)))
    b1 = b1 - W1[:HID].sum(axis=0)
    # wet4 rows carry W_et on BOTH pair rows (2f, 2f+1): the ctx matmul sums
    # the two slot-pair partials the transpose lands on adjacent rows.
    wet4 = np.zeros((128, 128), dtype=BF)
    bet4 = np.zeros((128, 1), dtype=np.float32)
    for g in range(4):
        for s1 in range(2):
            wet4[32 * g + s1:32 * g + 2 * EF + s1:2, 32 * g:32 * g + HID] = \
                W_et.astype(BF)
        bet4[32 * g:32 * g + HID, 0] = b_et
    cb16 = np.zeros((128, 512), dtype=BF)
    cb16[:, 0:128] = wet4
    cb16[:, 128:256] = np.tile(W1[:HID], (4, 1)).astype(BF)
    cb16[:, 256:384] = W1[HID:].astype(BF)
    cb16[:, 384:512] = W2.astype(BF)
    cf32 = np.zeros((128, 3), dtype=np.float32)
    cf32[:, 0:1] = bet4
    cf32[:, 1:2] = b1.reshape(NF, 1)
    cf32[:, 2:3] = b2.reshape(NF, 1)
    for dv in range(NCORES):
        in_maps[dv]["cb16"] = cb16.copy()
        in_maps[dv]["cf32"] = cf32.copy()
    return in_maps


def _unshard(results, plan):
    node_dev, node_lrow = plan["node_dev"], plan["node_lrow"]
    blocks = plan["blocks"]
    out = np.empty((N_NODES, NF), dtype=np.float32)
    for dv in range(NCORES):
        sel = node_dev == dv
        nid = np.nonzero(sel)[0]
        lr = node_lrow[sel]
        out_dev = results[dv]["outT"].T.astype(np.float32)
        out[nid] = out_dev[_mlpcol(lr, blocks)]
    return out


def _build_kernel(plan):
    kclasses = plan["kclasses"]
    R = plan["R"]
    blocks = plan["blocks"]
    n_blocks = len(blocks)
    zts = plan["zero_tail_start"]

    nc = bacc.Bacc("TRN2", target_bir_lowering=False, debug=False,
                   num_devices=NCORES)

    lg_offs = []
    acc = 0
    for (d, npad, off) in kclasses:
        T = (npad + 127) // 128
        lg_offs.append(acc)
        acc += T * d
    lg_total = acc

    ef_d = []
    for idx, (d, npad, off) in enumerate(kclasses):
        T = (npad + 127) // 128
        ef_d.append(nc.dram_tensor(f"ef{idx}", [128, T * d * EF], bf16,
                                   kind="ExternalInput"))
    c0 = lg_offs[1] if len(kclasses) > 1 else lg_total
    lg0_d = nc.dram_tensor("lg0", [128, c0], bf16, kind="ExternalInput")
    lgrest_d = nc.dram_tensor("lgrest", [128, lg_total - c0], bf16,
                              kind="ExternalInput")
    nfT_d = nc.dram_tensor("nfT", [NF, R], bf16, kind="ExternalInput")
    cb16_d = nc.dram_tensor("cb16", [128, 512], bf16, kind="ExternalInput")
    cf32_d = nc.dram_tensor("cf32", [128, 3], f32, kind="ExternalInput")
    out_d = nc.dram_tensor("outT", [NF, R], bf16, kind="ExternalOutput")

    # column base of each block in the [NF, R] layout
    col_base = []
    acc = 0
    for (t0b, ntb) in blocks:
        col_base.append(acc)
        acc += 4 * 32 * ntb
    assert acc == R

    with tile.TileContext(nc) as tc, ExitStack() as ctx:
        const_pool = ctx.enter_context(tc.tile_pool(name="const", bufs=1))
        agg_pool = ctx.enter_context(tc.tile_pool(name="agg", bufs=1))
        cls_pool = ctx.enter_context(tc.tile_pool(name="cls", bufs=4))
        work_pool = ctx.enter_context(tc.tile_pool(name="work", bufs=3))
        tree_pool = ctx.enter_context(tc.tile_pool(name="tree", bufs=3))
        small_pool = ctx.enter_context(tc.tile_pool(name="small", bufs=4))
        mlp_pool = ctx.enter_context(tc.tile_pool(name="mlp", bufs=3))
        ctx_pool = ctx.enter_context(tc.tile_pool(name="ctxs", bufs=2))
        ctx_psum = ctx.enter_context(tc.tile_pool(name="ctxp", bufs=2, space="PSUM"))
        mlp1_psum = ctx.enter_context(tc.tile_pool(name="m1p", bufs=4, space="PSUM"))
        mlp2_psum = ctx.enter_context(tc.tile_pool(name="m2p", bufs=2, space="PSUM"))

        # Two logits DMAs (class 0's small slice lands first so exp0 starts
        # ASAP) and all exps queued at the head of the Act engine's in-order
        # queue.
        lgt = cls_pool.tile([128, lg_total], bf16, name="lgall")
        nc.sync.dma_start(lgt[0:128, 0:c0], lg0_d.ap())
        nc.sync.dma_start(lgt[0:128, c0:lg_total], lgrest_d.ap())
        x_tiles = []
        for ci, (dc, npadc, _) in enumerate(kclasses):
            Tc = (npadc + 127) // 128
            xt = cls_pool.tile([128, Tc * dc], bf16, tag=f"x{ci}", name=f"x{ci}")
            nc.scalar.activation(xt[:], lgt[0:128, lg_offs[ci]:lg_offs[ci] + Tc * dc],
                                 mybir.ActivationFunctionType.Exp)
            x_tiles.append(xt)
        prefetched = {}

        def prefetch_ef(pi):
            if pi >= len(kclasses) or pi in prefetched:
                return
            dp, npadp, _ = kclasses[pi]
            Tp = (npadp + 127) // 128
            eftp = cls_pool.tile([128, Tp * dp * EF], bf16, tag="ef", name=f"eft{pi}")
            nc.sync.dma_start(eftp[:], ef_d[pi].ap())
            prefetched[pi] = eftp

        prefetch_ef(0)
        prefetch_ef(1)

        consts = const_pool.tile([128, 512], bf16, name="cb16c")
        nc.gpsimd.dma_start(consts[:], cb16_d.ap())
        constf = const_pool.tile([128, 3], f32, name="cf32c")
        nc.gpsimd.dma_start(constf[:], cf32_d.ap())
        wet4 = consts[0:128, 0:128]
        w1c = consts[0:128, 128:256]
        w1n = consts[0:128, 256:384]
        w2 = consts[0:128, 384:512]
        bet4 = constf[0:128, 0:1]
        b1 = constf[0:128, 1:2]
        b2 = constf[0:128, 2:3]

        aggT_blocks = []
        for b, (t0b, ntb) in enumerate(blocks):
            ab = agg_pool.tile([128, 32 * ntb], bf16, name=f"aggT{b}")
            aggT_blocks.append(ab)
        # deg0 tail rows (none for this input, but keep correct in general):
        # their aggT columns are never written by any class; zero them so the
        # ctx matmul sees context 0 (b_et==0 -> elu path produces cb'=1).
        if zts < R:
            t_lo = zts // 128
            for b, (t0b, ntb) in enumerate(blocks):
                lo = max(t0b, t_lo)
                hi = t0b + ntb
                if lo < hi:
                    nc.gpsimd.memset(
                        aggT_blocks[b][0:128, 32 * (lo - t0b):32 * (hi - t0b)], 0.0)

        # ---------------- Phase B+C: ELU + MLP per block ----------------
        nf_tiles = {}

        def prefetch_nf(b):
            if b >= n_blocks or b in nf_tiles:
                return
            t0b, ntb = blocks[b]
            W = 32 * ntb
            cb4 = col_base[b]
            t = mlp_pool.tile([NF, 4 * W], bf16, tag="nfblk", name=f"nfblk{b}")
            nc.sync.dma_start(t[:], nfT_d.ap()[:, cb4:cb4 + 4 * W])
            nf_tiles[b] = t

        cb_tiles = {}

        def emit_elu(b):
            t0b, ntb = blocks[b]
            W = 32 * ntb
            with tc.high_priority(offset=PRIO_OFF):
                ctx4 = ctx_psum.tile([128, W], f32, tag="ctx4", name=f"ctx4_{b}")
                nc.tensor.matmul(ctx4[:], wet4, aggT_blocks[b][:])
                s1 = mlp_pool.tile([128, W], bf16, tag="s1", name=f"s1_{b}")
                if b >= n_blocks - V_RELU_BLOCKS:
                    nc.vector.tensor_scalar(s1[:], ctx4[:], bet4, 0.0,
                                            mybir.AluOpType.add,
                                            mybir.AluOpType.max)
                else:
                    nc.scalar.activation(s1[:], ctx4[:],
                                         mybir.ActivationFunctionType.Relu,
                                         bias=bet4, scale=1.0)
                s2 = mlp_pool.tile([128, W], bf16, tag="s2", name=f"s2_{b}")
                nc.scalar.activation(s2[:], ctx4[:],
                                     mybir.ActivationFunctionType.Exp,
                                     bias=bet4, scale=1.0)
            # cb' = elu(ctx)+1 = min(exp(z),1) + relu(z); the +1 shift is
            # compensated in b1 on the host. s1/s2 ride bf16 so the stst
            # qualifies for the DVE's 2x 16-bit mode.
            cb = ctx_pool.tile([128, W], bf16, tag="cb", name=f"cb_{b}")
            with tc.high_priority(offset=PRIO_OFF):
                nc.vector.scalar_tensor_tensor(cb[:], s2[:], 1.0, s1[:],
                                               mybir.AluOpType.min,
                                               mybir.AluOpType.add)
            cb_tiles[b] = cb
            prefetch_nf(b)
            prefetch_nf(b + 1)

        def emit_mlp(b):
            t0b, ntb = blocks[b]
            W = 32 * ntb
            cb4 = col_base[b]
            cb = cb_tiles.pop(b)
            nfblk = nf_tiles.pop(b)
            oblk = mlp_pool.tile([NF, 4 * W], bf16, tag="oblk", name=f"oblk{b}")
            last = b == n_blocks - 1
            with tc.high_priority(offset=PRIO_OFF):
                for g in range(4):
                    j = 4 * b + g
                    nfb = nfblk[0:NF, g * W:(g + 1) * W]
                    ps1 = mlp1_psum.tile([NF, W], f32, tag="ps1", name=f"ps1_{j}")
                    cbg = cb[32 * g:32 * (g + 1), :]
                    w1cg = w1c[32 * g:32 * (g + 1), 0:NF]
                    nc.tensor.matmul(ps1[:], w1n, nfb, start=True, stop=False)
                    # base partition 96 is legal for a 32-row tile but the
                    # default inference path refuses it; pass it explicitly
                    nc.tensor.matmul(ps1[:], w1cg, cbg, start=False, stop=True,
                                     tile_position=(32 * g, 0))
                    h = mlp_pool.tile([NF, W], bf16, tag="h", name=f"h{j}")
                    tail = b >= n_blocks - V_RELU_BLOCKS
                    if tail and j % 2 == 1:
                        nc.vector.tensor_scalar(h[:], ps1[:], b1, 0.0,
                                                mybir.AluOpType.add,
                                                mybir.AluOpType.max)
                    else:
                        nc.scalar.activation(h[:], ps1[:],
                                             mybir.ActivationFunctionType.Relu,
                                             bias=b1, scale=1.0)
                    ps2 = mlp2_psum.tile([NF, W], f32, tag="ps2", name=f"ps2_{j}")
                    nc.tensor.matmul(ps2[:], w2, h[:])
                    ov = oblk[0:NF, g * W:(g + 1) * W]
                    if tail and j % 2 == 0:
                        nc.vector.tensor_scalar(ov, ps2[:], b2, 0.0,
                                                mybir.AluOpType.add,
                                                mybir.AluOpType.max)
                    else:
                        nc.scalar.activation(ov, ps2[:],
                                             mybir.ActivationFunctionType.Relu,
                                             bias=b2, scale=1.0)
            if last:
                nc.gpsimd.dma_start(out_d.ap()[:, cb4:cb4 + 2 * W],
                                    oblk[0:NF, 0:2 * W])
                nc.gpsimd.dma_start(out_d.ap()[:, cb4 + 2 * W:cb4 + 4 * W],
                                    oblk[0:NF, 2 * W:4 * W])
            else:
                nc.gpsimd.dma_start(out_d.ap()[:, cb4:cb4 + 4 * W], oblk[:])

        next_block = [0]

        # ------------- Phase A: per-class segment softmax + aggregation ------
        # Two-stage software pipeline: stage1 (den/recip/alpha/prod/tree) of
        # class i+1 is ISSUED before stage2 (transpose/emit) of class i, so
        # the in-order Vector queue always holds independent work ahead of
        # any op that waits on a GpSimd tree.
        stage_state = {}

        # The LAST class's prod+tree ride GpSimd (chunked so the
        # high-priority alphas still preempt): its den is hoisted early by
        # the scheduler, GpSimd computes it during its idle mid-window, and
        # the Vector engine reaches the final transposes sooner.
        n_cls = len(kclasses)

        def on_gpsimd(idx):
            # measured: offloading even the smallest tail class to GpSimd
            # shifts the ef DMA order and the V tail right; net negative
            return False

        alphas = {}

        def prep(idx):
            """den -> recip -> alpha for one class (idempotent)."""
            if idx in alphas:
                return alphas[idx]
            d, npad, off = kclasses[idx]
            T = (npad + 127) // 128
            xt = x_tiles[idx]
            x3 = xt[:].rearrange("p (t d) -> p t d", t=T)
            # den[p,t] = sum_d x; alpha = x * (1/den) pre-normalizes so the
            # tree output feeds the transpose directly. alpha rides the
            # otherwise-idle GpSimd (SBUF-only op); recip is DVE-only.
            den = small_pool.tile([128, T], f32, tag="den", name=f"den{idx}")
            nc.vector.tensor_reduce(den[:], x3, mybir.AxisListType.X,
                                    mybir.AluOpType.add)
            rd = small_pool.tile([128, T], f32, tag="rd", name=f"rd{idx}")
            nc.vector.reciprocal(rd[:], den[:])
            atag = f"gal{idx}" if on_gpsimd(idx) else "al"
            alpha = small_pool.tile([128, T * d], bf16, tag=atag, name=f"al{idx}")
            alpha3 = alpha[:].rearrange("p (t d) -> p t d", t=T)
            rd_b = rd[:].unsqueeze(2).broadcast_to([128, T, d])
            # class 0 is the latency-critical warmup chain: keep its alpha on
            # the DVE (no cross-engine hop); later classes ride GpSimd at
            # high priority so queued cb work never delays the prod chain
            if idx == 0:
                nc.vector.tensor_tensor(alpha3, x3, rd_b, mybir.AluOpType.mult)
            else:
                with tc.high_priority(offset=PRIO_OFF):
                    nc.gpsimd.tensor_tensor(alpha3, x3, rd_b,
                                            mybir.AluOpType.mult)
            alphas[idx] = alpha
            return alpha

        def stage1(idx):
            d, npad, off = kclasses[idx]
            d2 = d // 2
            T = (npad + 127) // 128
            prefetch_ef(idx + 2)
            eft = prefetched[idx]
            alpha = prep(idx)

            # prod[p, ts2, f, s1] = ef * alpha  (bf16 2x mode: packed pairs).
            # A small leading tile-slice rides GpSimd (slow but idle) so the
            # Vector engine finishes stage1 sooner.
            ptag = f"gprod{idx}" if on_gpsimd(idx) else "prod"
            prod = work_pool.tile([128, T * d * EF], bf16, tag=ptag, name=f"prod{idx}")
            ef4 = eft[:].rearrange("p (ts f s1) -> p ts f s1", f=EF, s1=2)
            al4 = alpha[:].rearrange("p (ts s1) -> p ts s1", s1=2) \
                .unsqueeze(2).broadcast_to([128, T * d2, EF, 2])
            prod4 = prod[:].rearrange("p (ts f s1) -> p ts f s1", f=EF, s1=2)
            gp = on_gpsimd(idx)

            def emit_tt(out_ap, a_ap, b_ap, n1, per_unit_free,
                        op=mybir.AluOpType.add):
                """tensor_tensor on V, or on G chunked along dim 1 so
                high-priority alphas can preempt between pieces."""
                if not gp:
                    nc.vector.tensor_tensor(out_ap, a_ap, b_ap, op)
                    return
                gsz = max(1, 640 // per_unit_free)
                nd = len(out_ap.shape)
                s = 0
                while s < n1:
                    e = min(n1, s + gsz)
                    if nd == 4:
                        sl = (slice(None), slice(s, e), slice(None), slice(None))
                    else:
                        sl = (slice(None), slice(s, e), slice(None))
                    nc.gpsimd.tensor_tensor(out_ap[sl], a_ap[sl], b_ap[sl], op)
                    s = e

            emit_tt(prod4, ef4, al4, T * d2, 32, mybir.AluOpType.mult)

            # pairwise-halving tree over s2 (32-elem contiguous runs at every
            # level); STOPS at slot-pairs -- the ctx matmul absorbs the last
            # add via the doubled wet4 rows.
            prodc = prod[:].rearrange("p (t s c) -> p t s c", t=T, c=32)
            agg_parts = []
            for bi, (s0, w) in enumerate(_bin_blocks(d2)):
                cur = prodc[:, :, s0:s0 + w, :]
                cw = w
                while cw > 1:
                    half = cw // 2
                    ttag = (f"gag{idx}_{bi}_{half}" if on_gpsimd(idx)
                            else f"ag{bi}_{half}")
                    nt = tree_pool.tile([128, T * half * 32], bf16,
                                        tag=ttag, name=f"ag{idx}_{bi}_{half}")
                    nt4 = nt[:].rearrange("p (t s c) -> p t s c", t=T, c=32)
                    emit_tt(nt4, cur[:, :, 0:half, :], cur[:, :, half:cw, :],
                            T, 32 * half)
                    cur = nt4
                    cw = half
                agg_parts.append(cur.rearrange("p t s c -> p t (s c)"))
            a16u = agg_parts[0]
            for k in range(1, len(agg_parts)):
                mtag = f"gam{idx}_{k}" if on_gpsimd(idx) else f"am{k}"
                ns = tree_pool.tile([128, T * 32], bf16, tag=mtag, name=f"am{idx}_{k}")
                ns3 = ns[:].rearrange("p (t c) -> p t c", c=32)
                emit_tt(ns3, a16u, agg_parts[k], T, 32)
                a16u = ns3
            stage_state[idx] = (a16u, T, off)

        # tile index -> block index (blocks may have irregular sizes)
        blk_of_tile = {}
        for b, (t0b, ntb) in enumerate(blocks):
            for tt in range(t0b, t0b + ntb):
                blk_of_tile[tt] = b

        def stage2(idx):
            a16u, T, off = stage_state.pop(idx)
            # 32x32 stream transpose into the aggT blocks: the 32-wide unit is
            # (f, s1) pairs -> aggT rows 2f+s1, matching wet4's doubled rows.
            a16t = a16u.tensor
            t0 = off // 128
            t = 0
            while t < T:
                b = blk_of_tile[t0 + t]
                t0b, ntb = blocks[b]
                te = min(T, t0b + ntb - t0)
                span = te - t
                in_ap = bass.AP(a16t, t * 32,
                                [[T * 32, 128], [32, span], [1, 32]])
                with tc.high_priority(offset=PRIO_OFF):
                    nc.vector.transpose(
                        aggT_blocks[b][0:128,
                                       32 * (t0 + t - t0b):32 * (t0 + te - t0b)],
                        in_ap)
                t = te
                # a block whose last tile was just transposed can start its
                # ELU+MLP now; this class's later transposes aren't its deps
                done_rows = 128 * (t0 + t)
                if idx == len(kclasses) - 1 and t == T:
                    done_rows = R * 2
                while (next_block[0] < n_blocks
                       and done_rows >= 128 * (blocks[next_block[0]][0]
                                               + blocks[next_block[0]][1])):
                    nb = next_block[0]
                    emit_elu(nb)
                    if nb > 0:
                        emit_mlp(nb - 1)
                    next_block[0] += 1

        # Issue the GpSimd-offloaded classes' den/recip/alpha and ef fetch
        # EARLY (priority = issue order): GpSimd computes their prod+tree in
        # its idle mid-window, off the Vector engine's critical path.
        for gidx in range(n_cls):
            if on_gpsimd(gidx):
                prefetch_ef(gidx)
                prep(gidx)

        # V-tree classes: transpose right after own tree (in-order V queue
        # anyway). G-tree classes: lag one class so V-side stage1 work covers
        # the GpSimd tree latency.
        pending = None
        for idx in range(len(kclasses)):
            stage1(idx)
            if pending is not None:
                stage2(pending)
                pending = None
            if on_gpsimd(idx) and idx + 1 < len(kclasses):
                pending = idx
            else:
                stage2(idx)
        if pending is not None:
            stage2(pending)
        emit_mlp(n_blocks - 1)

    nc.compile()
    return nc


def kernel(**inputs):
    dst = np.asarray(inputs["dst"])
    plan = _build_plan(dst)
    in_maps = _shard_inputs(inputs, plan)
    nc = _build_kernel(plan)
    trace = bool(int(os.environ.get("GNN_PROFILE", "0")))
    if trace:
        try:
            _install_ntff_hook()
        except Exception:
            pass
    res = run_bass_kernel_spmd(nc, in_maps, core_ids=list(range(NCORES)),
                               trace=trace)
    kernel.last_results = res
    return _unshard(res.results, plan)


def _install_ntff_hook():
    """Recreate antenv.axon_hooks (absent in this image) so
    run_bass_kernel_spmd(trace=True) can NTFF-profile via libaxon_pjrt.so."""
    import contextlib, ctypes, sys, types
    if 'antenv.axon_hooks' in sys.modules:
        return
    lib = ctypes.CDLL('/opt/axon/libaxon_pjrt.so')
    lib.axon_start_nrt_profile.argtypes = [ctypes.POINTER(ctypes.c_int64), ctypes.c_size_t]
    lib.axon_start_nrt_profile.restype = ctypes.c_int64
    lib.axon_stop_nrt_profile.argtypes = [ctypes.c_char_p]
    lib.axon_stop_nrt_profile.restype = ctypes.c_int64

    @contextlib.contextmanager
    def _hook(output_dir, device_ids):
        import jax
        jax.devices()
        if device_ids:
            ids = (ctypes.c_int64 * len(device_ids))(*device_ids)
            rc = lib.axon_start_nrt_profile(ids, len(device_ids))
        else:
            rc = lib.axon_start_nrt_profile(None, 0)
        if rc != 0:
            raise RuntimeError(f"axon_start_nrt_profile rc={rc}")
        try:
            yield
        finally:
            n = lib.axon_stop_nrt_profile(str(output_dir).encode())
            print(f"ntff profile: {n} file(s) written to {output_dir}", file=sys.stderr)

    mod = types.ModuleType('antenv.axon_hooks')
    mod.get_axon_ntff_profile_hook = lambda: _hook
    mod.set_axon_ntff_profile_hook = lambda h: None
    import antenv
    antenv.axon_hooks = mod
    sys.modules['antenv.axon_hooks'] = mod
